# revision 1
# baseline (speedup 1.0000x reference)
"""Causal self-attention (GQA + RoPE + QK-norm) Trainium2 Bass kernel.

Sharding: 8 cores = 4 batches x 2 head-groups.  Core c -> batch c//2,
q heads (c%2)*8..+8, kv heads (c%2)*2..+2.  wproj is row-sharded, so each
core emits a partial (T, C) output; the host sums the two partials per batch.

Device-side layout strategy (per core):
  - x is fed pre-transposed (xT, [C, T]) and bf16-cast by the host.
  - QKV projections produce Q,K token-major ([tok, cols]); RoPE + rms-norm
    run token-major (free-axis per-head reductions), then 128x128 PE
    transposes produce qT/kT feature-major for the attention matmuls.
    V is produced token-major, which is exactly the p@v stationary layout.
  - scores are computed transposed (scoresT[tk, tq]) so that after exp the
    p tiles are already the moving operand for the p@v matmul; the softmax
    denominator comes from a ones-column matmul accumulated in PSUM.
  - exp has no max-subtraction: qk-norm bounds |s| <= sqrt(128) ~ 11.32.
  - output projection accumulates over the 8 local heads; partial written
    fp32 to DRAM.
"""

import numpy as np
import ml_dtypes
from contextlib import ExitStack

import concourse.bass as bass
import concourse.mybir as mybir
import concourse.tile as tile
from concourse import bacc
from concourse.bass_utils import run_bass_kernel_spmd
from concourse.masks import make_identity

BF16 = mybir.dt.bfloat16
F32 = mybir.dt.float32
F32R = mybir.dt.float32r
AF = mybir.ActivationFunctionType

B, T, C = 4, 2048, 2048
H, KV, D = 16, 4, 128
HG, KVG = H // 2, KV // 2          # per-core q heads (8), kv heads (2)
QC, KC = HG * D, KVG * D           # 1024, 256
P = 128
TOKCH = T // P                     # 16 token chunks
NREP = H // KV                     # 4
EPS = 1e-5
NEG = -1.0e5                       # additive causal mask (exp -> 0)


DEBUG_DUMP = False
PHASES = ("A", "B", "C")


def _build():
    nc = bacc.Bacc("TRN2", target_bir_lowering=False, debug=False, num_devices=8)
    # x pre-tiled by the host as [tokch, p, co, tk] so every DMA partition row
    # is 4KB contiguous (co*tk*2B) instead of 256B strided
    xt = nc.dram_tensor("xt", [TOKCH, P, C // P, P], BF16, kind="ExternalInput")
    wq = nc.dram_tensor("wq", [C, QC], BF16, kind="ExternalInput")
    wkv = nc.dram_tensor("wkv", [C, 2 * KC], BF16, kind="ExternalInput")
    wp = nc.dram_tensor("wp", [QC, C], BF16, kind="ExternalInput")
    # cos/sin pre-tiled by host as [p, tc, d] (contiguous 4KB rows)
    cosd = nc.dram_tensor("cosd", [P, TOKCH, D // 2], F32, kind="ExternalInput")
    sind = nc.dram_tensor("sind", [P, TOKCH, D // 2], F32, kind="ExternalInput")
    out = nc.dram_tensor("out", [T, C], F32, kind="ExternalOutput")
    if DEBUG_DUMP:
        d_qt = nc.dram_tensor("d_qt", [P, HG, T], F32, kind="ExternalOutput")
        d_kt = nc.dram_tensor("d_kt", [P, KVG, T], F32, kind="ExternalOutput")
        d_v = nc.dram_tensor("d_v", [P, TOKCH, KC], F32, kind="ExternalOutput")
        d_yt = nc.dram_tensor("d_yt", [P, HG, T], F32, kind="ExternalOutput")

    with tile.TileContext(nc) as tc, ExitStack() as ctx:
        singles = ctx.enter_context(tc.tile_pool(name="singles", bufs=1))
        # bufs must cover the V-lag window (xtile(t) is re-read by the lagged
        # V projection at iteration t+VLAG); the pool closes with phase A
        phase_a_pools = ExitStack()
        xpool = phase_a_pools.enter_context(tc.tile_pool(name="xa", bufs=8))

        # ---- prefetch the first x tile before the weight bulk so the PE
        # can start within a few us ----
        # ---- resident tensors ----
        # weight DMAs issued per-co round-robin over both HWDGE queues so
        # early co chunks land in consumption order and issue rate isn't
        # limited by one sequencer (~600ns per dma_start).  The first x
        # chunk + first co weights go out first so the PE starts ASAP.
        wq_sb = singles.tile([P, C // P, QC], BF16)
        wkv_sb = singles.tile([P, C // P, 2 * KC], BF16)
        wqr = wq.rearrange("(co p) q -> p co q", p=P)
        wkvr = wkv.rearrange("(co p) q -> p co q", p=P)
        cos_sb = singles.tile([P, TOKCH, D // 2], F32)
        sin_sb = singles.tile([P, TOKCH, D // 2], F32)
        # DMAs ordered by first consumption: iteration 0 runs three column
        # passes (q0 cols 0:512, q1 cols 512:1024, k) so its first rope --
        # the start of the 160us serialized DVE chain that bounds phase A --
        # only needs the q0 half of wq plus xtile0.
        xtile0 = xpool.tile([P, C // P, P], BF16, tag="xt")
        nc.sync.dma_start(xtile0[:, 0:4, :], xt[0, :, 0:4, :])
        qs = [nc.sync, nc.scalar]
        # first 2 co's full weight needs, then the k half (it plus the first
        # x chunks gates the k rope, the start of the serialized DVE chain)
        nc.scalar.dma_start(wq_sb[:, 0:2, 0:512], wqr[:, 0:2, 0:512])
        nc.sync.dma_start(wq_sb[:, 0:2, 512:1024], wqr[:, 0:2, 512:1024])
        for i, co in enumerate(range(0, C // P, 4)):
            qs[i % 2].dma_start(wkv_sb[:, co:co + 4, 0:KC],
                                wkvr[:, co:co + 4, 0:KC])
        for g4 in range(1, 4):
            nc.sync.dma_start(xtile0[:, 4 * g4:4 * (g4 + 1), :],
                              xt[0, :, 4 * g4:4 * (g4 + 1), :])
        nc.scalar.dma_start(cos_sb, cosd[:])
        nc.scalar.dma_start(sin_sb, sind[:])
        for i, co in enumerate(range(2, C // P, 2)):
            qs[i % 2].dma_start(wq_sb[:, co:co + 2, 0:512],
                                wqr[:, co:co + 2, 0:512])
        for i, co in enumerate(range(2, C // P, 2)):
            qs[i % 2].dma_start(wq_sb[:, co:co + 2, 512:1024],
                                wqr[:, co:co + 2, 512:1024])
        # V weights stream last (first consumed at iteration VLAG, ~70us in)
        for co in range(0, C // P, 4):
            nc.gpsimd.dma_start(wkv_sb[:, co:co + 4, KC:2 * KC],
                                wkvr[:, co:co + 4, KC:2 * KC])

        ident = singles.tile([P, P], BF16)
        make_identity(nc, ident)
        ones_col = singles.tile([P, 1], BF16)
        nc.vector.memset(ones_col, 1.0)
        zero_col = singles.tile([P, 1], F32)
        nc.vector.memset(zero_col, 0.0)
        eps_col = singles.tile([P, 1], F32)
        nc.vector.memset(eps_col, EPS)
        nc.const_aps.aps[(F32, 0.0)] = zero_col[:]
        nc.const_aps.aps[(F32, EPS)] = eps_col[:]
        # scratch for the dummy exp that prewarms the exp act-table at the
        # A->B phase boundary (overlaps the 1.28us table load)
        warm = singles.tile([1, 1], F32)
        # k-psum evacuation buffers (parity-alternated): the single-buffered
        # k psum is freed by a quick Act copy instead of being held through
        # the whole rope-k chain, so the next iteration's k matmuls never WAR
        evk_a = singles.tile([P, KC], F32)
        evk_b = singles.tile([P, KC], F32)

        # diagonal-block mask: keep where i >= j (j = tk partition, i = tq
        # free).  bf16 so it can be ADDED into the scores psum by a 128-col
        # matmul (ident.T @ mask) instead of a DVE op in the exp chain.
        mask_sb = singles.tile([P, P], BF16)
        nc.vector.memset(mask_sb, 0.0)
        nc.gpsimd.affine_select(
            out=mask_sb, in_=mask_sb,
            compare_op=mybir.AluOpType.is_ge, fill=NEG,
            base=0, pattern=[[1, P]], channel_multiplier=-1,
        )

        qT = singles.tile([P, HG, T], BF16)      # [d, h, tok]
        kT = singles.tile([P, KVG, T], BF16)
        v_sb = singles.tile([P, TOKCH, KC], BF16)  # [tok%128, chunk, vcol]
        yT = singles.tile([P, HG, T], BF16)

        # ================= phase A: QKV proj + RoPE + qk-norm =============
        if "A" not in PHASES:
            pass
        else:
         with phase_a_pools, \
             tc.tile_pool(name="pa", bufs=2, space="PSUM") as pps, \
             tc.tile_pool(name="pkv", bufs=1, space="PSUM") as pkv, \
             tc.tile_pool(name="sa", bufs=3) as spool:
            # The V projection is split out of the QK pass and lagged by VLAG
            # chunks: the final VLAG V-chunks are pure PE work that runs while
            # the last rope chains (DVE) drain, so phase B starts without
            # waiting on the phase-A tail.
            VLAG = 6
            nco = C // P
            xtiles = {}
            # transposes lag one iteration behind their rope chain so they
            # never sit dep-blocked in the PE's 4-deep wait queue
            pending_tr = []

            def flush_trs():
                while pending_tr:
                    qbf, dstT, h0, nh, tt = pending_tr.pop(0)
                    pst = pkv.tile([P, 4, P], BF16, tag="tr")
                    for i in range(nh):
                        nc.tensor.transpose(pst[:, i, :], qbf[:, i, :], ident)
                    nc.scalar.copy(
                        dstT[:, h0:h0 + nh, tt * P:(tt + 1) * P], pst[:, 0:nh, :])

            def v_chunk(tv):
                xv = xtiles.pop(tv)
                # alternate psum tags so consecutive V chunks don't serialize
                # on one buffer's Act-copy release
                ps_v = pkv.tile([P, KC], F32, tag=("v" if tv % 2 == 0 else "v2"))
                for co in range(nco):
                    nc.tensor.matmul(ps_v, xv[:, co, :],
                                     wkv_sb[:, co, KC:2 * KC],
                                     start=(co == 0), stop=(co == nco - 1))
                # cast straight to resident token-major buffer (Act engine;
                # DVE is the critical engine in this phase)
                nc.scalar.copy(v_sb[:, tv, :], ps_v)

            for t in range(TOKCH):
                if t == 0:
                    xtile = xtile0
                else:
                    xtile = xpool.tile([P, C // P, P], BF16, tag="xt")
                    nc.sync.dma_start(xtile, xt[t])
                xtiles[t] = xtile
                ps_q0 = pps.tile([P, 512], F32, tag="q0")
                ps_q1 = pps.tile([P, 512], F32, tag="q1")
                ps_k = pkv.tile([P, KC], F32, tag="k")

                # Q/K: fused multi-head rope + rms-norm + cast + transpose
                def rope_norm(ps, nh, dstT, h0, qscale, rsq_dve=False):
                    h2 = D // 2
                    v4 = ps.rearrange("p (h a d) -> p h a d", h=nh, a=2)
                    q1, q2 = v4[:, :, 0, :], v4[:, :, 1, :]
                    r = spool.tile([P, nh, 2, h2], F32, tag=f"rope{nh}")
                    r1, r2 = r[:, :, 0, :], r[:, :, 1, :]
                    s2 = spool.tile([P, nh, h2], F32, tag=f"scr{nh}")
                    cs = cos_sb[:, t, None, :].to_broadcast([P, nh, h2])
                    sn = sin_sb[:, t, None, :].to_broadcast([P, nh, h2])
                    nc.vector.tensor_mul(r1, q1, cs)
                    nc.vector.tensor_mul(s2, q2, sn)
                    nc.vector.tensor_sub(r1, r1, s2)
                    nc.vector.tensor_mul(r2, q1, sn)
                    nc.vector.tensor_mul(s2, q2, cs)
                    nc.vector.tensor_add(r2, r2, s2)
                    rf = r.rearrange("p h a d -> p h (a d)")
                    sq = spool.tile([P, nh, D], F32, tag=f"sq{nh}")
                    ss = spool.tile([P, nh], F32, tag=f"ss{nh}")
                    if rsq_dve:
                        # keep the last iteration's rope entirely off the Act
                        # engine so phase B's first exps aren't queued behind it
                        nc.vector.tensor_mul(sq, rf, rf)
                    else:
                        nc.scalar.activation(sq, rf, AF.Square)
                    nc.vector.tensor_reduce(ss, sq, axis=mybir.AxisListType.X,
                                            op=mybir.AluOpType.add)
                    rq = spool.tile([P, nh], F32, tag=f"rq{nh}")
                    if rsq_dve:
                        # DVE-only fast inverse sqrt (bit trick + 2 Newton
                        # steps, qscale folded into the last).  Used for the
                        # final token chunk so the previous iteration's Sqrt
                        # is the Act engine's last sqrt-set op and the exp
                        # table load hides behind the V tail.
                        ALU = mybir.AluOpType
                        I32 = mybir.dt.int32
                        fx = spool.tile([P, nh], F32, tag=f"fx{nh}")
                        nc.vector.tensor_scalar(fx, ss, 1.0 / D, EPS,
                                                op0=ALU.mult, op1=ALU.add)
                        fj = spool.tile([P, nh], I32, tag=f"fj{nh}")
                        nc.vector.tensor_scalar(fj, fx[:].bitcast(I32), 1, None,
                                                op0=ALU.logical_shift_right)
                        nc.vector.tensor_scalar(fj, fj, -1, 0x5f3759df + 1,
                                                op0=ALU.bitwise_xor, op1=ALU.add)
                        fy = fj[:].bitcast(F32)
                        fa = spool.tile([P, nh], F32, tag=f"fa{nh}")
                        nc.vector.tensor_mul(fa, fy, fy)
                        nc.vector.tensor_mul(fa, fa, fx)
                        nc.vector.tensor_scalar(fa, fa, -0.5, 1.5,
                                                op0=ALU.mult, op1=ALU.add)
                        nc.vector.tensor_mul(rq, fy, fa)
                        nc.vector.tensor_mul(fa, rq, rq)
                        nc.vector.tensor_mul(fa, fa, fx)
                        nc.vector.tensor_scalar(fa, fa, -0.5 * qscale,
                                                1.5 * qscale,
                                                op0=ALU.mult, op1=ALU.add)
                        nc.vector.tensor_mul(rq, rq, fa)
                    else:
                        rt = spool.tile([P, nh], F32, tag=f"rt{nh}")
                        nc.scalar.activation(rt, ss, AF.Sqrt, scale=1.0 / D,
                                             bias=EPS)
                        nc.vector.reciprocal(rq, rt)
                        if qscale != 1.0:
                            nc.vector.tensor_scalar_mul(rq, rq, qscale)
                    qbf = spool.tile([P, nh, D], BF16, tag=f"qbf{nh}")
                    nc.vector.tensor_mul(qbf, rf, rq[:, :, None].to_broadcast([P, nh, D]))
                    pending_tr.append((qbf, dstT, h0, nh, t))

                qsc = 1.0 / float(np.sqrt(D))
                for co in range(nco):
                    lhsT = xtile[:, co, :]
                    st = dict(start=(co == 0), stop=(co == nco - 1))
                    nc.tensor.matmul(ps_q0, lhsT, wq_sb[:, co, 0:512], **st)
                    nc.tensor.matmul(ps_q1, lhsT, wq_sb[:, co, 512:1024], **st)
                    nc.tensor.matmul(ps_k, lhsT, wkv_sb[:, co, 0:KC], **st)
                evk = evk_a if t % 2 == 0 else evk_b
                nc.scalar.copy(evk, ps_k)
                rope_norm(ps_q0, 4, qT, 0, qsc)
                rope_norm(ps_q1, 4, qT, 4, qsc)
                rope_norm(evk[:], KVG, kT, 0, 1.0)
                if t >= VLAG:
                    v_chunk(t - VLAG)
                # previous iteration's transposes: rope chains long done
                flush_trs()
                if t == TOKCH - 1:
                    # prewarm the exp act-table; the 1.28us load runs behind
                    # the V tail
                    nc.scalar.activation(warm, zero_col[0:1, :], AF.Exp)

            # lagged V tail: pure PE work that covers the final rope chains;
            # the last transposes flush once their rope chain has had V cover
            for tv in range(TOKCH - VLAG, TOKCH):
                v_chunk(tv)
                if tv == TOKCH - 2:
                    flush_trs()

        # ================= phase B: attention ============================
        # wp prefetch: issue at phase-B start so the tiles are resident long
        # before phase C begins (phase-A pools have closed, SBUF is free)
        wpool = ctx.enter_context(tc.tile_pool(name="wp", bufs=1))
        wpr = wp.rearrange("(hc p) c -> p hc c", p=P)
        wp_ts = []
        for ct in range(C // 512):
            wp_t = wpool.tile([P, HG, 512], BF16, tag=f"wpt{ct}")
            nc.sync.dma_start(wp_t, wpr[:, :, ct * 512:(ct + 1) * 512])
            wp_ts.append(wp_t)

        if "B" not in PHASES:
            pass
        else:
         with tc.tile_pool(name="psc", bufs=4, space="PSUM") as psc, \
             tc.tile_pool(name="psy", bufs=2, space="PSUM") as psy, \
             tc.tile_pool(name="pss", bufs=2, space="PSUM") as pss, \
             tc.tile_pool(name="pb", bufs=6) as ppool, \
             tc.tile_pool(name="sb", bufs=3) as bpool:
            NT = T // 512  # 4 tq tiles
            # software pipeline: the PE queue is in-order, so scores for
            # chunk idx+DEPTH are emitted before pv/ones of chunk idx; the
            # scores->mask->exp chain (~1.6us) hides behind DEPTH chunks of
            # PE work.  The (h, c) stream is flattened so the pipeline also
            # covers head boundaries.
            DEPTH = 4
            for t in range(NT):
                nch = 4 * (t + 1)
                items = [(h, c) for h in range(HG) for c in range(nch)]
                live = {}

                def front(idx):
                    h, c = items[idx]
                    g = h // NREP
                    o = c * P - t * 512
                    col0 = max(o, 0)
                    ps_sc = psc.tile([P, 512], F32, tag="sc")
                    nc.tensor.matmul(
                        ps_sc[:, col0:512], kT[:, g, c * P:(c + 1) * P],
                        qT[:, h, t * 512 + col0:(t + 1) * 512],
                        start=True, stop=(o < 0))
                    if o >= 0:
                        # after the col0 shift the partial block is always the
                        # i' >= j triangle; accumulate the additive mask with
                        # a 128-col matmul (53ns) right behind the scores
                        nc.tensor.matmul(ps_sc[:, col0:col0 + P], ident,
                                         mask_sb, start=False, stop=True)
                    pt = ppool.tile([P, 512], BF16, tag="pt")
                    nc.scalar.activation(pt[:, col0:512], ps_sc[:, col0:512],
                                         AF.Exp)
                    live[idx] = (pt, col0)

                for i in range(min(DEPTH, len(items))):
                    front(i)
                ys = {}
                for idx, (h, c) in enumerate(items):
                    if idx + DEPTH < len(items):
                        front(idx + DEPTH)
                    g = h // NREP
                    if c == 0:
                        ps_y = psy.tile([P, 512], F32, tag="y")
                        ps_sden = pss.tile([P, 512], F32, tag="sden")
                        ys[h] = (ps_y, ps_sden)
                    ps_y, ps_sden = ys[h]
                    ps_s = ps_sden[0:1, :]
                    pt, col0 = live.pop(idx)
                    st = dict(start=(c == 0), stop=(c == nch - 1))
                    nc.tensor.matmul(ps_y[:, col0:512],
                                     v_sb[:, c, g * P:(g + 1) * P],
                                     pt[:, col0:512], **st)
                    nc.tensor.matmul(ps_s[:, col0:512], ones_col,
                                     pt[:, col0:512], **st)
                    if c == nch - 1:
                        # recip first (frees the single pss buffer fastest),
                        # then copy (frees ps_y); normalize the bf16 slice in
                        # place on the Pool engine once the broadcast lands
                        yslice = yT[:, h, t * 512:(t + 1) * 512]
                        rc = bpool.tile([1, 512], F32, tag="rc")
                        nc.vector.reciprocal(rc, ps_s)
                        nc.vector.tensor_copy(yslice, ps_y)
                        rb = bpool.tile([P, 512], F32, tag="rb")
                        nc.gpsimd.partition_broadcast(rb, rc, channels=P)
                        nc.vector.tensor_mul(yslice, yslice, rb)

        if DEBUG_DUMP:
            with tc.tile_pool(name="dbg", bufs=2) as dpool:
                for h in range(HG):
                    dt_ = dpool.tile([P, T], F32, tag="d")
                    nc.vector.tensor_copy(dt_, qT[:, h, :])
                    nc.sync.dma_start(d_qt[:, h, :], dt_)
                    dt_ = dpool.tile([P, T], F32, tag="d")
                    nc.vector.tensor_copy(dt_, yT[:, h, :])
                    nc.sync.dma_start(d_yt[:, h, :], dt_)
                for g in range(KVG):
                    dt_ = dpool.tile([P, T], F32, tag="d")
                    nc.vector.tensor_copy(dt_, kT[:, g, :])
                    nc.sync.dma_start(d_kt[:, g, :], dt_)
                dt_ = dpool.tile([P, TOKCH * KC], F32, tag="d")
                nc.vector.tensor_copy(dt_.rearrange("p (a b) -> p a b", a=TOKCH), v_sb[:, :, :])
                nc.sync.dma_start(d_v[:, :, :], dt_.rearrange("p (a b) -> p a b", a=TOKCH))

        # ================= phase C: output projection =====================
        if "C" not in PHASES:
            pass
        else:
         with tc.tile_pool(name="po", bufs=2, space="PSUM") as pso, \
             tc.tile_pool(name="so", bufs=3) as opool:
            for t in range(TOKCH):
                for ct in range(C // 512):
                    ps_o = pso.tile([P, 512], F32, tag="o")
                    last = (t == TOKCH - 1 and ct == C // 512 - 1)
                    ob = opool.tile([P, 512], F32, tag="ob")
                    if not last:
                        for hc in range(HG):
                            nc.tensor.matmul(
                                ps_o, yT[:, hc, t * P:(t + 1) * P],
                                wp_ts[ct][:, hc, :],
                                start=(hc == 0), stop=(hc == HG - 1))
                        nc.vector.tensor_copy(ob, ps_o)
                        nc.sync.dma_start(
                            out[t * P:(t + 1) * P, ct * 512:(ct + 1) * 512], ob)
                    else:
                        # final tile in two pipelined halves to shorten the
                        # copy->dma drain tail; halves go out on different
                        # queues so the transfers overlap
                        for q in range(2):
                            cs = slice(q * 256, (q + 1) * 256)
                            for hc in range(HG):
                                nc.tensor.matmul(
                                    ps_o[:, cs], yT[:, hc, t * P:(t + 1) * P],
                                    wp_ts[ct][:, hc, cs],
                                    start=(hc == 0), stop=(hc == HG - 1))
                            (nc.vector.tensor_copy if q == 0
                             else nc.scalar.copy)(ob[:, cs], ps_o[:, cs])
                            (nc.sync if q == 0 else nc.scalar).dma_start(
                                out[t * P:(t + 1) * P,
                                    ct * 512 + q * 256:ct * 512 + (q + 1) * 256],
                                ob[:, cs])
    nc.compile()
    return nc


_NC_CACHE = []


def _get_prog():
    if not _NC_CACHE:
        _NC_CACHE.append(_build())
    return _NC_CACHE[0]


def _make_in_maps(inputs):
    x, cos, sin = inputs["x"], inputs["cos"], inputs["sin"]
    wq, wk, wv, wproj = inputs["wq"], inputs["wk"], inputs["wv"], inputs["wproj"]
    bf = ml_dtypes.bfloat16
    # [p, tc, d] tiling (contiguous 4KB DMA rows)
    cos2 = np.ascontiguousarray(
        cos.reshape(TOKCH, P, D // 2).transpose(1, 0, 2), dtype=np.float32)
    sin2 = np.ascontiguousarray(
        sin.reshape(TOKCH, P, D // 2).transpose(1, 0, 2), dtype=np.float32)
    in_maps = []
    for core in range(8):
        b, g = core // 2, core % 2
        qs = slice(g * QC, (g + 1) * QC)
        ks = slice(g * KC, (g + 1) * KC)
        # x[b].T is [C, T]; tile to [tokch, p(C-chunk), co, tk]
        xtb = (x[b].T.astype(bf)
               .reshape(C // P, P, TOKCH, P)     # [co, p, tc, tk]
               .transpose(2, 1, 0, 3))           # [tc, p, co, tk]
        in_maps.append({
            "xt": np.ascontiguousarray(xtb),
            "wq": np.ascontiguousarray(wq[:, qs]).astype(bf),
            "wkv": np.ascontiguousarray(np.hstack([wk[:, ks], wv[:, ks]])).astype(bf),
            "wp": np.ascontiguousarray(wproj[qs, :]).astype(bf),
            "cosd": cos2,
            "sind": sin2,
        })
    return in_maps


def kernel(x, cos, sin, wq, wk, wv, wproj):
    nc = _get_prog()
    in_maps = _make_in_maps(dict(x=x, cos=cos, sin=sin, wq=wq, wk=wk, wv=wv, wproj=wproj))
    res = run_bass_kernel_spmd(nc, in_maps, core_ids=list(range(8))).results
    outp = np.empty((B, T, C), np.float32)
    for b in range(B):
        outp[b] = res[2 * b]["out"] + res[2 * b + 1]["out"]
    return outp



# revision 39
# speedup vs baseline: 1.1821x; 1.1821x over previous
"""Causal self-attention (GQA + RoPE + QK-norm) Trainium2 Bass kernel.

Sharding: 8 cores = 4 batches x 2 head-groups.  Core c -> batch c//2,
q heads (c%2)*8..+8, kv heads (c%2)*2..+2.  wproj is row-sharded, so each
core emits a partial (T, C) output; the host sums the two partials per batch.

Device-side layout strategy (per core):
  - x and the projection weights ship as error-compensated fp8 hi/lo pairs
    (hi = fp8(v), lo = fp8(v - hi)); the QKV and output projections run as
    3-pass DoubleRow fp8 matmuls (hi*hi + hi*lo + lo*hi, dropping the
    ~0.4%-scale lo*lo term) -- 4/3x faster than bf16 on the PE.
  - QKV projections produce Q,K token-major.  rms-norm runs BEFORE rope
    (rope is a rotation per frequency pair, so it preserves the per-head
    norm): the psum evacuation applies the per-head 1/rms as the Act-copy
    scale, then the rope runs as an all-bf16 DVE chain (2x mode), and the
    128x128 PE transposes produce qT/kT feature-major.  V is token-major
    fp16, which is exactly the p@v stationary layout.
  - scores are computed transposed (scoresT[tk, tq]) in bf16, psums in
    2-chunk pairs so one exp covers 1024 columns; exp applies bias -4 and
    writes fp16 p-tiles (fits fp16 range since qk-norm bounds |s|<=11.32;
    the bias cancels between numerator and denominator).
  - softmax denominator: fp16 pair-sums on the DVE + quad-sums on the Pool
    engine feed a ones-column matmul per quad -- the PE streams den at 1/4
    of the direct cost.
  - output projection: yT is normalized and split into fp8 hi/lo during
    phase B; 3-pass fp8 DoubleRow over hc pairs.  Partial written fp32.
"""

import numpy as np
import ml_dtypes
from contextlib import ExitStack

import concourse.bass as bass
import concourse.mybir as mybir
import concourse.tile as tile
from concourse import bacc
from concourse.bass_utils import run_bass_kernel_spmd
from concourse.masks import make_identity

BF16 = mybir.dt.bfloat16
F16 = mybir.dt.float16
F32 = mybir.dt.float32
F8E4 = mybir.dt.float8e4
DR = mybir.MatmulPerfMode.DoubleRow
AF = mybir.ActivationFunctionType
ALU = mybir.AluOpType

B, T, C = 4, 2048, 2048
H, KV, D = 16, 4, 128
HG, KVG = H // 2, KV // 2          # per-core q heads (8), kv heads (2)
QC, KC = HG * D, KVG * D           # 1024, 256
P = 128
TOKCH = T // P                     # 16 token chunks
NREP = H // KV                     # 4
EPS = 1e-5
NEG = -1.0e5                       # additive causal mask (exp -> 0)
WS = 32.0                          # fp8 weight pre-scale (host): keeps w
                                   # entries in e4m3 normal range; q/k are
                                   # scale-invariant through rms-norm, v is
                                   # rescaled by 1/WS at psum evacuation
YS = 8.0                           # y pre-scale for the fp8 hi/lo split of
                                   # the output-projection LHS
EXPB = -4.0                        # exp(s-4) so p fits fp16 range; cancels
                                   # between numerator and denominator


DEBUG_DUMP = False
PHASES = ("A", "B", "C")


def _build():
    nc = bacc.Bacc("TRN2", target_bir_lowering=False, debug=False, num_devices=8)
    # x pre-tiled by the host as [tokch, p, co, tk] so every DMA partition row
    # is contiguous; hi/lo fp8 pair (combined traffic = bf16 baseline)
    xth = nc.dram_tensor("xth", [TOKCH, P, C // P, P], F8E4, kind="ExternalInput")
    xtl = nc.dram_tensor("xtl", [TOKCH, P, C // P, P], F8E4, kind="ExternalInput")
    wqh = nc.dram_tensor("wqh", [C, QC], F8E4, kind="ExternalInput")
    wql = nc.dram_tensor("wql", [C, QC], F8E4, kind="ExternalInput")
    wkvh = nc.dram_tensor("wkvh", [C, 2 * KC], F8E4, kind="ExternalInput")
    wkvl = nc.dram_tensor("wkvl", [C, 2 * KC], F8E4, kind="ExternalInput")
    wph = nc.dram_tensor("wph", [QC, C], F8E4, kind="ExternalInput")
    wpl = nc.dram_tensor("wpl", [QC, C], F8E4, kind="ExternalInput")
    # cos/sin pre-tiled by host as [p, tc, d] bf16 (keeps the rope chain in
    # the DVE 2x mode)
    cosd = nc.dram_tensor("cosd", [P, TOKCH, D // 2], BF16, kind="ExternalInput")
    sind = nc.dram_tensor("sind", [P, TOKCH, D // 2], BF16, kind="ExternalInput")
    out = nc.dram_tensor("out", [T, C], F32, kind="ExternalOutput")

    with tile.TileContext(nc) as tc, ExitStack() as ctx:
        singles = ctx.enter_context(tc.tile_pool(name="singles", bufs=1))
        # bufs must cover the V-lag window (xtile(t) is re-read by the lagged
        # V projection at iteration t+VLAG); the pool closes with phase A
        phase_a_pools = ExitStack()
        xpool = phase_a_pools.enter_context(tc.tile_pool(name="xa", bufs=8))

        # ---- resident tensors ----
        wq_h = singles.tile([P, C // P, QC], F8E4)
        wq_l = singles.tile([P, C // P, QC], F8E4)
        wkv_h = singles.tile([P, C // P, 2 * KC], F8E4)
        wkv_l = singles.tile([P, C // P, 2 * KC], F8E4)
        wqhr = wqh.rearrange("(co p) q -> p co q", p=P)
        wqlr = wql.rearrange("(co p) q -> p co q", p=P)
        wkvhr = wkvh.rearrange("(co p) q -> p co q", p=P)
        wkvlr = wkvl.rearrange("(co p) q -> p co q", p=P)
        cos_sb = singles.tile([P, TOKCH, D // 2], BF16)
        sin_sb = singles.tile([P, TOKCH, D // 2], BF16)
        # x hi/lo tiles for iteration 0 go out first (they gate the PE),
        # then weights in consumption order (first q pair, k halves, q bulk,
        # V last -- first consumed at iteration VLAG)
        xtile0h = xpool.tile([P, C // P, P], F8E4, tag="xh")
        xtile0l = xpool.tile([P, C // P, P], F8E4, tag="xl")
        nc.sync.dma_start(xtile0h[:, 0:4, :], xth[0, :, 0:4, :])
        nc.scalar.dma_start(xtile0l[:, 0:4, :], xtl[0, :, 0:4, :])
        # weight DMAs in exact consumption order (co-major, hi then lo, q
        # columns before k) spread over four queues so the t=0 matmul
        # stream never waits long on one HWDGE sequencer
        qs4 = [nc.sync, nc.scalar, nc.gpsimd]
        qi = 0

        def wdma(dst, src):
            nonlocal_qi = wdma.qi
            qs4[nonlocal_qi % 3].dma_start(dst, src)
            wdma.qi = nonlocal_qi + 1
        wdma.qi = 0
        for co in range(0, C // P, 4):
            wdma(wq_h[:, co:co + 4, 0:512], wqhr[:, co:co + 4, 0:512])
            wdma(wq_l[:, co:co + 4, 0:512], wqlr[:, co:co + 4, 0:512])
        nc.sync.dma_start(xtile0h[:, 4:16, :], xth[0, :, 4:16, :])
        nc.scalar.dma_start(xtile0l[:, 4:16, :], xtl[0, :, 4:16, :])
        for co in range(0, C // P, 8):
            wdma(wq_h[:, co:co + 8, 512:1024], wqhr[:, co:co + 8, 512:1024])
            wdma(wq_l[:, co:co + 8, 512:1024], wqlr[:, co:co + 8, 512:1024])
        for co in range(0, C // P, 8):
            wdma(wkv_h[:, co:co + 8, 0:KC], wkvhr[:, co:co + 8, 0:KC])
            wdma(wkv_l[:, co:co + 8, 0:KC], wkvlr[:, co:co + 8, 0:KC])
        # x tiles for t=1,2 ahead of the V-weight bulk
        xtile_pre = {}
        for tp in (1, 2):
            xph = xpool.tile([P, C // P, P], F8E4, tag="xh")
            xpl = xpool.tile([P, C // P, P], F8E4, tag="xl")
            nc.sync.dma_start(xph, xth[tp])
            nc.scalar.dma_start(xpl, xtl[tp])
            xtile_pre[tp] = (xph, xpl)
        nc.gpsimd.dma_start(cos_sb, cosd[:])
        nc.gpsimd.dma_start(sin_sb, sind[:])
        for co in range(0, C // P, 4):
            nc.gpsimd.dma_start(wkv_h[:, co:co + 4, KC:2 * KC],
                                wkvhr[:, co:co + 4, KC:2 * KC])
            nc.gpsimd.dma_start(wkv_l[:, co:co + 4, KC:2 * KC],
                                wkvlr[:, co:co + 4, KC:2 * KC])

        ident = singles.tile([P, P], BF16)
        make_identity(nc, ident)
        # ones MATRIX: stationary for the denominator matmuls -> the psum
        # output carries the tk-sum replicated across all 128 partitions
        # (same cost as a 1-row output; kills the partition_broadcast)
        ones_mat = singles.tile([P, P], F16)
        nc.vector.memset(ones_mat, 1.0)
        zero_col = singles.tile([P, 1], F32)
        nc.vector.memset(zero_col, 0.0)
        eps_col = singles.tile([P, 1], F32)
        nc.vector.memset(eps_col, EPS)
        nb_col = singles.tile([P, 1], F32)
        nc.vector.memset(nb_col, EXPB)
        nc.const_aps.aps[(F32, 0.0)] = zero_col[:]
        nc.const_aps.aps[(F32, EPS)] = eps_col[:]
        nc.const_aps.aps[(F32, EXPB)] = nb_col[:]
        # scratch for the dummy exp that prewarms the exp act-table at the
        # A->B phase boundary (overlaps the 1.28us table load)
        warm = singles.tile([1, 1], F32)

        # diagonal-block mask: keep where i >= j (j = tk partition, i = tq
        # free).  bf16 so it can be ADDED into the scores psum by a 128-col
        # matmul (ident.T @ mask) instead of a DVE op in the exp chain.
        mask_sb = singles.tile([P, P], BF16)
        nc.vector.memset(mask_sb, 0.0)
        nc.gpsimd.affine_select(
            out=mask_sb, in_=mask_sb,
            compare_op=ALU.is_ge, fill=NEG,
            base=0, pattern=[[1, P]], channel_multiplier=-1,
        )

        qT = singles.tile([P, HG, T], BF16)      # [d, h, tok]
        kT = singles.tile([P, KVG, T], BF16)
        v_sb = singles.tile([P, TOKCH, KC], F16)  # [tok%128, chunk, vcol]
        # output-projection LHS, fp8 hi/lo split of yT*YS (written in B)
        y8h = singles.tile([P, HG, T], F8E4)
        y8l = singles.tile([P, HG, T], F8E4)

        # ================= phase A: QKV proj + norm + RoPE ================
        if "A" not in PHASES:
            pass
        else:
         with phase_a_pools, \
             tc.tile_pool(name="pa", bufs=2, space="PSUM") as pps, \
             tc.tile_pool(name="pkv", bufs=1, space="PSUM") as pkv, \
             tc.tile_pool(name="sa", bufs=3) as spool:
            VLAG = 6
            nco = C // P
            xtiles = {}
            # transposes lag one iteration behind their rope chain so they
            # never sit dep-blocked in the PE's 4-deep wait queue
            pending_tr = []

            def flush_trs():
                while pending_tr:
                    qbf, dstT, h0, nh, tt = pending_tr.pop(0)
                    pst = pkv.tile([P, 4, P], BF16, tag="tr")
                    for i in range(nh):
                        nc.tensor.transpose(pst[:, i, :], qbf[:, i, :], ident)
                    nc.scalar.copy(
                        dstT[:, h0:h0 + nh, tt * P:(tt + 1) * P], pst[:, 0:nh, :])

            def mm3(ps, xh, xl, wh, wl, cols):
                # 3-pass hi/lo fp8 DoubleRow accumulation, co-major so the
                # weight consumption follows the startup DMA stream order
                for co in range(0, nco, 2):
                    for pi, (a, b) in enumerate(((xh, wh), (xh, wl), (xl, wh))):
                        nc.tensor.matmul(
                            ps, a[:, co:co + 2, :], b[:, co:co + 2, cols],
                            start=(pi == 0 and co == 0),
                            stop=(pi == 2 and co == nco - 2),
                            perf_mode=DR)

            def v_chunk(tv):
                xvh, xvl = xtiles.pop(tv)
                ps_v = pkv.tile([P, KC], F32, tag="v")
                mm3(ps_v, xvh, xvl, wkv_h, wkv_l, slice(KC, 2 * KC))
                # cast straight to resident token-major fp16 buffer; 1/WS
                # undoes the host-side fp8 weight pre-scale
                nc.scalar.mul(v_sb[:, tv, :], ps_v, 1.0 / WS)

            for t in range(TOKCH):
                if t == 0:
                    xtileh, xtilel = xtile0h, xtile0l
                elif t in xtile_pre:
                    xtileh, xtilel = xtile_pre[t]
                else:
                    xtileh = xpool.tile([P, C // P, P], F8E4, tag="xh")
                    xtilel = xpool.tile([P, C // P, P], F8E4, tag="xl")
                    nc.sync.dma_start(xtileh, xth[t])
                    nc.scalar.dma_start(xtilel, xtl[t])
                xtiles[t] = (xtileh, xtilel)
                ps_q0 = pps.tile([P, 512], F32, tag="q0")
                ps_q1 = pps.tile([P, 512], F32, tag="q1")
                ps_k = pkv.tile([P, KC], F32, tag=("k" if t % 2 == 0 else "k2"))

                # rms-norm stats run off the psum (pre-rope: rope is a
                # rotation per frequency pair so it preserves per-head
                # norms); the rope chain itself runs on an all-bf16
                # evacuation (DVE 2x mode), and the 1/rms multiplier is
                # applied once at the end.
                def norm_rope(ps, nh, dstT, h0, qscale):
                    h2 = D // 2
                    psv = ps.rearrange("p (h d) -> p h d", h=nh)
                    sq = spool.tile([P, nh, D], BF16, tag=f"sq{h0}_{nh}")
                    nc.scalar.activation(sq, psv, AF.Square)
                    ss = spool.tile([P, nh], BF16, tag=f"ss{h0}_{nh}")
                    with nc.allow_low_precision(
                            reason="rms stats; 0.4% scale err is fine"):
                        nc.vector.tensor_reduce(ss, sq,
                                                axis=mybir.AxisListType.X,
                                                op=ALU.add)
                    rt = spool.tile([P, nh], F32, tag=f"rt{h0}_{nh}")
                    nc.scalar.activation(rt, ss, AF.Sqrt, scale=1.0 / D,
                                         bias=EPS)
                    rq = spool.tile([P, nh], F32, tag=f"rq{h0}_{nh}")
                    nc.vector.reciprocal(rq, rt)
                    if qscale != 1.0:
                        nc.vector.tensor_scalar_mul(rq, rq, qscale)
                    qe = spool.tile([P, nh, D], BF16, tag=f"qe{h0}_{nh}")
                    nc.scalar.copy(qe, psv)
                    # rope on the bf16 evacuation (all-2-byte DVE ops)
                    q1, q2 = qe[:, :, 0:h2], qe[:, :, h2:D]
                    r = spool.tile([P, nh, 2, h2], BF16, tag=f"ro{h0}_{nh}")
                    r1, r2 = r[:, :, 0, :], r[:, :, 1, :]
                    s2 = spool.tile([P, nh, h2], BF16, tag=f"sc{h0}_{nh}")
                    cs = cos_sb[:, t, None, :].to_broadcast([P, nh, h2])
                    sn = sin_sb[:, t, None, :].to_broadcast([P, nh, h2])
                    nc.vector.tensor_mul(r1, q1, cs)
                    nc.vector.tensor_mul(s2, q2, sn)
                    nc.vector.tensor_sub(r1, r1, s2)
                    nc.vector.tensor_mul(r2, q1, sn)
                    nc.vector.tensor_mul(s2, q2, cs)
                    nc.vector.tensor_add(r2, r2, s2)
                    rf = r.rearrange("p h a d -> p h (a d)")
                    qbf = spool.tile([P, nh, D], BF16, tag=f"qb{h0}_{nh}")
                    nc.vector.tensor_mul(
                        qbf, rf, rq[:, :, None].to_broadcast([P, nh, D]))
                    pending_tr.append((qbf, dstT, h0, nh, t))

                qsc = 1.0 / float(np.sqrt(D))
                mm3(ps_q0, xtileh, xtilel, wq_h, wq_l, slice(0, 512))
                mm3(ps_q1, xtileh, xtilel, wq_h, wq_l, slice(512, 1024))
                mm3(ps_k, xtileh, xtilel, wkv_h, wkv_l, slice(0, KC))
                norm_rope(ps_q0, 4, qT, 0, qsc)
                norm_rope(ps_q1, 4, qT, 4, qsc)
                norm_rope(ps_k, KVG, kT, 0, 1.0)
                if t >= VLAG:
                    v_chunk(t - VLAG)
                # previous iteration's transposes: rope chains long done
                flush_trs()
                if t == TOKCH - 1:
                    # prewarm the exp act-table; the 1.28us load runs behind
                    # the V tail
                    nc.scalar.activation(warm, zero_col[0:1, :], AF.Exp)

            # lagged V tail: pure PE work that covers the final rope chains
            for tv in range(TOKCH - VLAG, TOKCH):
                v_chunk(tv)
                if tv == TOKCH - 2:
                    flush_trs()

        # ================= phase B: attention ============================
        # wp prefetch: issue at phase-B start so the tiles are resident long
        # before phase C begins (phase-A pools have closed, SBUF is free)
        wpool = ctx.enter_context(tc.tile_pool(name="wp", bufs=1))
        wphr = wph.rearrange("(hc p) c -> p hc c", p=P)
        wplr = wpl.rearrange("(hc p) c -> p hc c", p=P)
        wp_ts = []
        for ct in range(C // 512):
            wp_th = wpool.tile([P, HG, 512], F8E4, tag=f"wpth{ct}")
            wp_tl = wpool.tile([P, HG, 512], F8E4, tag=f"wptl{ct}")
            nc.sync.dma_start(wp_th, wphr[:, :, ct * 512:(ct + 1) * 512])
            nc.scalar.dma_start(wp_tl, wplr[:, :, ct * 512:(ct + 1) * 512])
            wp_ts.append((wp_th, wp_tl))

        if "B" not in PHASES:
            pass
        else:
         with tc.tile_pool(name="psc", bufs=2, space="PSUM") as psc, \
             tc.tile_pool(name="psy", bufs=2, space="PSUM") as psy, \
             tc.tile_pool(name="pss", bufs=1, space="PSUM") as pss, \
             tc.tile_pool(name="po", bufs=1, space="PSUM") as pso, \
             tc.tile_pool(name="pb", bufs=6) as ppool, \
             tc.tile_pool(name="sb", bufs=4) as bpool, \
             tc.tile_pool(name="so", bufs=3) as opool:
            NT = T // 512  # 4 tq tiles
            OSC = 1.0 / (YS * WS)
            cq = []  # pending output-projection tiles, emitted at head
            #          boundaries to fill the PE while psum chains drain
            ci = 0

            def emit_c(tc_, ct, last=False, alt=False):
                nonlocal ci
                wp_th, wp_tl = wp_ts[ct]
                # in the tail (alt=True) the psy ring is idle: alternate
                # between the pso bank and psy's so back-to-back projection
                # tiles don't serialize on one psum evacuation
                if alt and ci % 2 == 0:
                    ps_o = psy.tile([P, 512], F32, tag="y")
                else:
                    ps_o = pso.tile([P, 512], F32, tag="o")
                ob = opool.tile([P, 512], F32, tag="ob")
                tsl = slice(tc_ * P, (tc_ + 1) * P)

                def proj(cs, ps):
                    for pi, (a, b) in enumerate(
                            ((y8h, wp_th), (y8h, wp_tl), (y8l, wp_th))):
                        for hc in range(0, HG, 2):
                            nc.tensor.matmul(
                                ps, a[:, hc:hc + 2, tsl],
                                b[:, hc:hc + 2, cs],
                                start=(pi == 0 and hc == 0),
                                stop=(pi == 2 and hc == HG - 2),
                                perf_mode=DR)

                if not last:
                    proj(slice(0, 512), ps_o)
                    ci += 1
                    if ci % 2 == 0:
                        nc.vector.tensor_scalar_mul(ob, ps_o, OSC)
                    else:
                        nc.scalar.mul(ob, ps_o, OSC)
                    (nc.sync if ci % 2 == 0 else nc.scalar).dma_start(
                        out[tc_ * P:(tc_ + 1) * P, ct * 512:(ct + 1) * 512], ob)
                else:
                    # final tile in two pipelined halves to shorten the
                    # copy->dma drain tail
                    for q in range(2):
                        cs = slice(q * 256, (q + 1) * 256)
                        proj(cs, ps_o[:, cs])
                        if q == 0:
                            nc.vector.tensor_scalar_mul(ob[:, cs],
                                                        ps_o[:, cs], OSC)
                        else:
                            nc.scalar.mul(ob[:, cs], ps_o[:, cs], OSC)
                        (nc.sync if q == 0 else nc.scalar).dma_start(
                            out[tc_ * P:(tc_ + 1) * P,
                                ct * 512 + q * 256:ct * 512 + (q + 1) * 256],
                            ob[:, cs])
            # software pipeline over chunk PAIRS: scores+exp for pair
            # idx+DEPTH are emitted before pv of pair idx, so the
            # scores->mask->exp chain hides behind PE work.
            DEPTH = 2
            # tile order: start with a mid-length tile so the first tile's
            # head boundaries are not too short, then feed each finished
            # tile's projection tiles into the next tile's head boundaries
            # (the short-head t=0/1 tiles get C-fill this way too)
            TORD = [2, 3, 1, 0]
            for ti, t in enumerate(TORD):
                nch = 4 * (t + 1)
                npair = nch // 2
                items = [(h, pr) for h in range(HG) for pr in range(npair)]
                live = {}
                # previously finished tq-tile's projection tiles
                if ti >= 1:
                    tprev = TORD[ti - 1]
                    cq.extend((tc_, ct) for tc_ in range(4 * tprev,
                                                         4 * tprev + 4)
                              for ct in range(C // 512))

                def front(idx):
                    h, pr = items[idx]
                    g = h // NREP
                    c0 = 2 * pr
                    ps_sc = psc.tile([P, 2, 512], F32, tag="sc")
                    pt = ppool.tile([P, 2, 512], F16, tag="pt")
                    col0s = []
                    for i, c in enumerate((c0, c0 + 1)):
                        o = c * P - t * 512
                        col0 = max(o, 0)
                        col0s.append(col0)
                        nc.tensor.matmul(
                            ps_sc[:, i, col0:512], kT[:, g, c * P:(c + 1) * P],
                            qT[:, h, t * 512 + col0:(t + 1) * 512],
                            start=True, stop=(o < 0))
                        if o >= 0:
                            # after the col0 shift the partial block is always
                            # the i' >= j triangle; accumulate the additive
                            # mask with a 128-col matmul right behind scores
                            nc.tensor.matmul(ps_sc[:, i, col0:col0 + P], ident,
                                             mask_sb, start=False, stop=True)
                    a0, a1 = col0s
                    # one exp instruction across both psum banks; for a
                    # diagonal pair the flat range [a0:1024] includes the
                    # stale segment [512:512+a1) -- exp of an old (finite)
                    # score lands in a pt region that nothing reads
                    nc.scalar.activation(
                        pt.rearrange("p a b -> p (a b)")[:, a0:1024],
                        ps_sc.rearrange("p a b -> p (a b)")[:, a0:1024],
                        AF.Exp, bias=EXPB)
                    # fp16 pair-sum for the denominator tree (DVE 2x); for
                    # the diagonal pairs only the region >= a0 is live, and
                    # [a0, a1) has just the first chunk
                    s01 = bpool.tile([P, 512], F16, tag="s01")
                    if a1 > a0:
                        nc.vector.tensor_copy(s01[:, a0:a1], pt[:, 0, a0:a1])
                        nc.vector.tensor_add(s01[:, a1:512], pt[:, 0, a1:512],
                                             pt[:, 1, a1:512])
                    else:
                        nc.vector.tensor_add(s01, pt[:, 0, :], pt[:, 1, :])
                    live[idx] = (pt, col0s, s01)

                for i in range(min(DEPTH, len(items))):
                    front(i)
                ys = {}
                s01s = {}
                for idx, (h, pr) in enumerate(items):
                    if idx + DEPTH < len(items):
                        front(idx + DEPTH)
                    g = h // NREP
                    c0 = 2 * pr
                    if pr == 0:
                        ps_y = psy.tile([P, 512], F32, tag="y")
                        ps_s = pss.tile([P, 512], F32, tag="s")
                        ys[h] = (ps_y, ps_s)
                    ps_y, ps_s = ys[h]
                    pt, col0s, s01 = live.pop(idx)
                    for i, c in enumerate((c0, c0 + 1)):
                        col0 = col0s[i]
                        nc.tensor.matmul(ps_y[:, col0:512],
                                         v_sb[:, c, g * P:(g + 1) * P],
                                         pt[:, i, col0:512],
                                         start=(pr == 0 and i == 0),
                                         stop=(pr == npair - 1 and i == 1))
                    # denominator tree: full pairs combine to quads on the
                    # Pool engine (one ones-matmul per quad); the two
                    # diagonal pairs feed the ones-matmul directly
                    nq = npair - 2  # full pairs; always even
                    if pr < nq:
                        if pr % 2 == 0:
                            s01s[h] = s01
                        else:
                            q4 = bpool.tile([P, 512], F16, tag="q4")
                            nc.gpsimd.tensor_tensor(q4, s01s.pop(h), s01,
                                                    ALU.add)
                            nc.tensor.matmul(ps_s, ones_mat, q4,
                                             start=(pr == 1),
                                             stop=False)
                    else:
                        a0 = col0s[0]
                        nc.tensor.matmul(ps_s[:, a0:512], ones_mat,
                                         s01[:, a0:512],
                                         start=(pr == nq and nq == 0),
                                         stop=(pr == npair - 1))
                    if pr == npair - 1:
                        # normalize + split into the fp8 hi/lo
                        # output-projection operand (den is already
                        # replicated across partitions by ones_mat)
                        rc = bpool.tile([P, 512], F32, tag="rc")
                        nc.vector.reciprocal(rc, ps_s)
                        yn = bpool.tile([P, 512], BF16, tag="yn")
                        nc.vector.scalar_tensor_tensor(
                            yn, ps_y, YS, rc, op0=ALU.mult, op1=ALU.mult)
                        yh = y8h[:, h, t * 512:(t + 1) * 512]
                        nc.vector.tensor_copy(yh, yn)
                        nc.vector.scalar_tensor_tensor(
                            y8l[:, h, t * 512:(t + 1) * 512], yn, 1.0, yh,
                            op0=ALU.mult, op1=ALU.subtract)
                        # fill the head-boundary psum-chain drain with two
                        # output-projection tiles of the previous tq-tile
                        for _ in range(2):
                            if cq:
                                tc_, ct = cq.pop(0)
                                emit_c(tc_, ct)

            # tail: the final processed tq-tile's projection tiles
            while cq:
                tc_, ct = cq.pop(0)
                emit_c(tc_, ct, alt=True)
            tlast = TORD[-1]
            ctail = [(tc_, ct) for tc_ in range(4 * tlast, 4 * tlast + 4)
                     for ct in range(C // 512)]
            for tc_, ct in ctail:
                emit_c(tc_, ct, alt=True,
                       last=((tc_, ct) == ctail[-1]))

    nc.compile()
    return nc


_NC_CACHE = []


def _get_prog():
    if not _NC_CACHE:
        _NC_CACHE.append(_build())
    return _NC_CACHE[0]


def _split8(a):
    e4 = ml_dtypes.float8_e4m3
    hi = a.astype(e4)
    lo = (a - hi.astype(np.float32)).astype(e4)
    return hi, lo


def _make_in_maps(inputs):
    x, cos, sin = inputs["x"], inputs["cos"], inputs["sin"]
    wq, wk, wv, wproj = inputs["wq"], inputs["wk"], inputs["wv"], inputs["wproj"]
    bf = ml_dtypes.bfloat16
    # [p, tc, d] tiling (contiguous DMA rows)
    cos2 = np.ascontiguousarray(
        cos.reshape(TOKCH, P, D // 2).transpose(1, 0, 2)).astype(bf)
    sin2 = np.ascontiguousarray(
        sin.reshape(TOKCH, P, D // 2).transpose(1, 0, 2)).astype(bf)
    in_maps = []
    for core in range(8):
        b, g = core // 2, core % 2
        qs = slice(g * QC, (g + 1) * QC)
        ks = slice(g * KC, (g + 1) * KC)
        # x[b].T is [C, T]; tile to [tokch, p(C-chunk), co, tk]
        xtb = (x[b].T.astype(np.float32)
               .reshape(C // P, P, TOKCH, P)     # [co, p, tc, tk]
               .transpose(2, 1, 0, 3))           # [tc, p, co, tk]
        xh, xl = _split8(xtb)
        wqh, wql = _split8(np.ascontiguousarray(wq[:, qs]) * WS)
        wkvh, wkvl = _split8(
            np.hstack([wk[:, ks], wv[:, ks]]) * WS)
        wph, wpl = _split8(np.ascontiguousarray(wproj[qs, :]) * WS)
        in_maps.append({
            "xth": np.ascontiguousarray(xh),
            "xtl": np.ascontiguousarray(xl),
            "wqh": np.ascontiguousarray(wqh),
            "wql": np.ascontiguousarray(wql),
            "wkvh": np.ascontiguousarray(wkvh),
            "wkvl": np.ascontiguousarray(wkvl),
            "wph": np.ascontiguousarray(wph),
            "wpl": np.ascontiguousarray(wpl),
            "cosd": cos2,
            "sind": sin2,
        })
    return in_maps


def kernel(x, cos, sin, wq, wk, wv, wproj):
    nc = _get_prog()
    in_maps = _make_in_maps(dict(x=x, cos=cos, sin=sin, wq=wq, wk=wk, wv=wv, wproj=wproj))
    res = run_bass_kernel_spmd(nc, in_maps, core_ids=list(range(8))).results
    outp = np.empty((B, T, C), np.float32)
    for b in range(B):
        outp[b] = res[2 * b]["out"] + res[2 * b + 1]["out"]
    return outp


# revision 76
# speedup vs baseline: 1.2257x; 1.0370x over previous
"""Causal self-attention (GQA + RoPE + QK-norm) Trainium2 Bass kernel.

Sharding: 8 cores = 4 batches x 2 head-groups.  Core c -> batch c//2,
q heads (c%2)*8..+8, kv heads (c%2)*2..+2.  wproj is row-sharded, so each
core emits a partial (T, C) output; the host sums the two partials per batch.

Device-side layout strategy (per core):
  - x and the projection weights ship as error-compensated fp8 hi/lo pairs
    (hi = fp8(v), lo = fp8(v - hi)); the QKV and output projections run as
    3-pass DoubleRow fp8 matmuls (hi*hi + hi*lo + lo*hi, dropping the
    ~0.4%-scale lo*lo term) -- 4/3x faster than bf16 on the PE.
  - QKV projections produce Q,K token-major.  rms-norm runs BEFORE rope
    (rope is a rotation per frequency pair, so it preserves the per-head
    norm): the psum evacuation applies the per-head 1/rms as the Act-copy
    scale, then the rope runs as an all-bf16 DVE chain (2x mode), and the
    128x128 PE transposes produce qT/kT feature-major.  V is token-major
    fp16, which is exactly the p@v stationary layout.
  - scores are computed transposed (scoresT[tk, tq]) in bf16, psums in
    2-chunk pairs so one exp covers 1024 columns; exp applies bias -4 and
    writes fp16 p-tiles (fits fp16 range since qk-norm bounds |s|<=11.32;
    the bias cancels between numerator and denominator).
  - softmax denominator: fp16 pair-sums on the DVE + quad-sums on the Pool
    engine feed a ones-column matmul per quad -- the PE streams den at 1/4
    of the direct cost.
  - output projection: yT is normalized and split into fp8 hi/lo during
    phase B; 3-pass fp8 DoubleRow over hc pairs.  Partial written fp32.
"""

import numpy as np
import ml_dtypes
from contextlib import ExitStack

import concourse.bass as bass
import concourse.mybir as mybir
import concourse.tile as tile
from concourse import bacc
from concourse.bass_utils import run_bass_kernel_spmd
from concourse.masks import make_identity

BF16 = mybir.dt.bfloat16
F16 = mybir.dt.float16
F32 = mybir.dt.float32
F8E4 = mybir.dt.float8e4
DR = mybir.MatmulPerfMode.DoubleRow
AF = mybir.ActivationFunctionType
ALU = mybir.AluOpType

B, T, C = 4, 2048, 2048
H, KV, D = 16, 4, 128
HG, KVG = H // 2, KV // 2          # per-core q heads (8), kv heads (2)
QC, KC = HG * D, KVG * D           # 1024, 256
P = 128
TOKCH = T // P                     # 16 token chunks
NREP = H // KV                     # 4
EPS = 1e-5
NEG = -1.0e5                       # additive causal mask (exp -> 0)
WS = 32.0                          # fp8 weight pre-scale (host): keeps w
                                   # entries in e4m3 normal range; q/k are
                                   # scale-invariant through rms-norm, v is
                                   # rescaled by 1/WS at psum evacuation
YS = 8.0                           # y pre-scale for the fp8 hi/lo split of
                                   # the output-projection LHS
EXPB = -4.0                        # exp(s-4) so p fits fp16 range; cancels
                                   # between numerator and denominator


DEBUG_DUMP = False
PHASES = ("A", "B", "C")


def _build():
    nc = bacc.Bacc("TRN2", target_bir_lowering=False, debug=False, num_devices=8)
    # x pre-tiled by the host as [tokch, p, co, tk] so every DMA partition row
    # is contiguous; hi/lo fp8 pair (combined traffic = bf16 baseline)
    # hi/lo fp8 pairs ship interleaved in one tensor per operand: one DMA
    # carries both (the HWDGE issue sequencer at ~625ns/DMA is the startup
    # bottleneck, so DMA count matters more than size)
    xt8 = nc.dram_tensor("xt8", [TOKCH, P, 2, C // P, P], F8E4,
                         kind="ExternalInput")
    # outer dims split by first use so every DMA slice stays <=3 dims:
    # wq8 by column half, wkv8 by K/V, wp8 by ct chunk
    wq8 = nc.dram_tensor("wq8", [2, C, 2, 512], F8E4, kind="ExternalInput")
    wkv8 = nc.dram_tensor("wkv8", [2, C, 2, KC], F8E4, kind="ExternalInput")
    wp8 = nc.dram_tensor("wp8", [4, QC, 2, 512], F8E4, kind="ExternalInput")
    # cos/sin pre-tiled by host as [p, tc, d] bf16 (keeps the rope chain in
    # the DVE 2x mode)
    cosd = nc.dram_tensor("cosd", [P, TOKCH, D // 2], BF16, kind="ExternalInput")
    sind = nc.dram_tensor("sind", [P, TOKCH, D // 2], BF16, kind="ExternalInput")
    out = nc.dram_tensor("out", [T, C], F32, kind="ExternalOutput")

    with tile.TileContext(nc) as tc, ExitStack() as ctx:
        singles = ctx.enter_context(tc.tile_pool(name="singles", bufs=1))
        # bufs must cover the V-lag window (xtile(t) is re-read by the lagged
        # V projection at iteration t+VLAG); the pool closes with phase A
        phase_a_pools = ExitStack()
        xpool = phase_a_pools.enter_context(tc.tile_pool(name="xa", bufs=8))

        # ---- resident tensors ----
        wq_sb = singles.tile([P, 2, C // P, 2, 512], F8E4)
        wkv_sb = singles.tile([P, 2, C // P, 2, KC], F8E4)
        wqr = wq8.rearrange("ch (co p) two q -> p ch co two q", p=P)
        wkvr = wkv8.rearrange("kv (co p) two q -> p kv co two q", p=P)
        cos_sb = singles.tile([P, TOKCH, D // 2], BF16)
        sin_sb = singles.tile([P, TOKCH, D // 2], BF16)
        # x hi/lo tiles for iteration 0 go out first (they gate the PE),
        # then weights in consumption order (first q pair, k halves, q bulk,
        # V last -- first consumed at iteration VLAG)
        xtile0 = xpool.tile([P, 2, C // P, P], F8E4, tag="xt")
        nc.sync.dma_start(xtile0[:, :, 0:4, :], xt8[0, :, :, 0:4, :])
        # weight DMAs in exact consumption order (co-major, q columns
        # before k); hi+lo ride together so the issue count stays low
        qs2 = [nc.sync, nc.scalar]
        for i, co in enumerate(range(0, C // P, 4)):
            qs2[i % 2].dma_start(wq_sb[:, 0, co:co + 4], wqr[:, 0, co:co + 4])
        nc.scalar.dma_start(xtile0[:, :, 4:16, :], xt8[0, :, :, 4:16, :])
        for i, co in enumerate(range(0, C // P, 8)):
            qs2[i % 2].dma_start(wq_sb[:, 1, co:co + 8], wqr[:, 1, co:co + 8])
        for i, co in enumerate(range(0, C // P, 8)):
            qs2[(i + 1) % 2].dma_start(wkv_sb[:, 0, co:co + 8],
                                       wkvr[:, 0, co:co + 8])
        # x tiles for t=1,2 ahead of the V-weight bulk
        xtile_pre = {}
        for tp in (1, 2):
            xp = xpool.tile([P, 2, C // P, P], F8E4, tag="xt")
            qs2[tp % 2].dma_start(xp, xt8[tp])
            xtile_pre[tp] = xp
        nc.gpsimd.dma_start(cos_sb, cosd[:])
        nc.gpsimd.dma_start(sin_sb, sind[:])

        ident = singles.tile([P, P], BF16)
        make_identity(nc, ident)
        # ones MATRIX: stationary for the denominator matmuls -> the psum
        # output carries the tk-sum replicated across all 128 partitions
        # (same cost as a 1-row output; kills the partition_broadcast)
        ones_mat = singles.tile([P, P], F16)
        nc.vector.memset(ones_mat, 1.0)
        zero_col = singles.tile([P, 1], F32)
        nc.vector.memset(zero_col, 0.0)
        eps_col = singles.tile([P, 1], F32)
        nc.vector.memset(eps_col, EPS)
        nb_col = singles.tile([P, 1], F32)
        nc.vector.memset(nb_col, EXPB)
        nc.const_aps.aps[(F32, 0.0)] = zero_col[:]
        nc.const_aps.aps[(F32, EPS)] = eps_col[:]
        nc.const_aps.aps[(F32, EXPB)] = nb_col[:]
        # scratch for the dummy exp that prewarms the exp act-table at the
        # A->B phase boundary (overlaps the 1.28us table load)
        warm = singles.tile([1, 1], F32)

        # diagonal-block mask: keep where i >= j (j = tk partition, i = tq
        # free).  bf16 so it can be ADDED into the scores psum by a 128-col
        # matmul (ident.T @ mask) instead of a DVE op in the exp chain.
        mask_sb = singles.tile([P, P], BF16)
        nc.vector.memset(mask_sb, 0.0)
        nc.gpsimd.affine_select(
            out=mask_sb, in_=mask_sb,
            compare_op=ALU.is_ge, fill=NEG,
            base=0, pattern=[[1, P]], channel_multiplier=-1,
        )

        qT = singles.tile([P, HG, T], BF16)      # [d, h, tok]
        kT = singles.tile([P, KVG, T], BF16)
        v_sb = singles.tile([P, TOKCH, KC], F16)  # [tok%128, chunk, vcol]
        # output-projection LHS, fp8 hi/lo split of yT*YS (written in B)
        y8h = singles.tile([P, HG, T], F8E4)
        y8l = singles.tile([P, HG, T], F8E4)

        # ================= phase A: QKV proj + norm + RoPE ================
        if "A" not in PHASES:
            pass
        else:
         with phase_a_pools, \
             tc.tile_pool(name="pa", bufs=2, space="PSUM") as pps, \
             tc.tile_pool(name="pkv", bufs=1, space="PSUM") as pkv, \
             tc.tile_pool(name="sa", bufs=3) as spool:
            VLAG = 6
            nco = C // P
            xtiles = {}
            # transposes lag one iteration behind their rope chain so they
            # never sit dep-blocked in the PE's 4-deep wait queue
            pending_tr = []

            def flush_trs(upto=None):
                while pending_tr and (upto is None or pending_tr[0][4] <= upto):
                    qbf, dstT, h0, nh, tt = pending_tr.pop(0)
                    pst = pkv.tile([P, 4, P], BF16, tag="tr")
                    for i in range(nh):
                        nc.tensor.transpose(pst[:, i, :], qbf[:, i, :], ident)
                    nc.scalar.copy(
                        dstT[:, h0:h0 + nh, tt * P:(tt + 1) * P], pst[:, 0:nh, :])

            def mm3(ps, xt, w_sb, half):
                # 3-pass hi/lo fp8 DoubleRow accumulation, co-major so the
                # weight consumption follows the startup DMA stream order
                for co in range(0, nco, 2):
                    for pi, (ai, bi) in enumerate(((0, 0), (0, 1), (1, 0))):
                        nc.tensor.matmul(
                            ps, xt[:, ai, co:co + 2, :],
                            w_sb[:, half, co:co + 2, bi, :],
                            start=(pi == 0 and co == 0),
                            stop=(pi == 2 and co == nco - 2),
                            perf_mode=DR)

            def v_chunk(tv):
                xv = xtiles.pop(tv)
                ps_v = pkv.tile([P, KC], F32, tag="v")
                mm3(ps_v, xv, wkv_sb, 1)
                # cast straight to resident token-major fp16 buffer; 1/WS
                # undoes the host-side fp8 weight pre-scale
                nc.scalar.mul(v_sb[:, tv, :], ps_v, 1.0 / WS)

            for t in range(TOKCH):
                if t == 0:
                    xtile = xtile0
                elif t in xtile_pre:
                    xtile = xtile_pre[t]
                else:
                    xtile = xpool.tile([P, 2, C // P, P], F8E4, tag="xt")
                    (nc.sync if t % 2 == 0 else nc.scalar).dma_start(
                        xtile, xt8[t])
                xtiles[t] = xtile
                ps_q0 = pps.tile([P, 512], F32, tag="q0")
                ps_q1 = pps.tile([P, 512], F32, tag="q1")
                ps_k = pkv.tile([P, KC], F32, tag=("k" if t % 2 == 0 else "k2"))

                # rms-norm stats run off the psum (pre-rope: rope is a
                # rotation per frequency pair so it preserves per-head
                # norms); the rope chain itself runs on an all-bf16
                # evacuation (DVE 2x mode), and the 1/rms multiplier is
                # applied once at the end.
                def norm_rope(ps, nh, dstT, h0, qscale):
                    h2 = D // 2
                    psv = ps.rearrange("p (h d) -> p h d", h=nh)
                    sq = spool.tile([P, nh, D], BF16, tag=f"sq{h0}_{nh}")
                    nc.scalar.activation(sq, psv, AF.Square)
                    ss = spool.tile([P, nh], BF16, tag=f"ss{h0}_{nh}")
                    with nc.allow_low_precision(
                            reason="rms stats; 0.4% scale err is fine"):
                        nc.vector.tensor_reduce(ss, sq,
                                                axis=mybir.AxisListType.X,
                                                op=ALU.add)
                    rt = spool.tile([P, nh], F32, tag=f"rt{h0}_{nh}")
                    nc.scalar.activation(rt, ss, AF.Sqrt, scale=1.0 / D,
                                         bias=EPS)
                    rq = spool.tile([P, nh], F32, tag=f"rq{h0}_{nh}")
                    nc.vector.reciprocal(rq, rt)
                    if qscale != 1.0:
                        nc.vector.tensor_scalar_mul(rq, rq, qscale)
                    qe = spool.tile([P, nh, D], BF16, tag=f"qe{h0}_{nh}")
                    nc.scalar.copy(qe, psv)
                    # rope on the bf16 evacuation (all-2-byte DVE ops)
                    q1, q2 = qe[:, :, 0:h2], qe[:, :, h2:D]
                    r = spool.tile([P, nh, 2, h2], BF16, tag=f"ro{h0}_{nh}")
                    r1, r2 = r[:, :, 0, :], r[:, :, 1, :]
                    s2 = spool.tile([P, nh, h2], BF16, tag=f"sc{h0}_{nh}")
                    cs = cos_sb[:, t, None, :].to_broadcast([P, nh, h2])
                    sn = sin_sb[:, t, None, :].to_broadcast([P, nh, h2])
                    nc.vector.tensor_mul(r1, q1, cs)
                    nc.vector.tensor_mul(s2, q2, sn)
                    nc.vector.tensor_sub(r1, r1, s2)
                    nc.vector.tensor_mul(r2, q1, sn)
                    nc.vector.tensor_mul(s2, q2, cs)
                    nc.vector.tensor_add(r2, r2, s2)
                    rf = r.rearrange("p h a d -> p h (a d)")
                    qbf = spool.tile([P, nh, D], BF16, tag=f"qb{h0}_{nh}")
                    nc.vector.tensor_mul(
                        qbf, rf, rq[:, :, None].to_broadcast([P, nh, D]))
                    pending_tr.append((qbf, dstT, h0, nh, t))

                qsc = 1.0 / float(np.sqrt(D))
                mm3(ps_q0, xtile, wq_sb, 0)
                mm3(ps_q1, xtile, wq_sb, 1)
                mm3(ps_k, xtile, wkv_sb, 0)
                norm_rope(ps_q0, 4, qT, 0, qsc)
                norm_rope(ps_q1, 4, qT, 4, qsc)
                norm_rope(ps_k, KVG, kT, 0, 1.0)
                if t == 2:
                    # V weights: first consumed at t=VLAG; issuing here keeps
                    # their transfers out of the startup-critical DMA window
                    for co in range(0, C // P, 4):
                        nc.gpsimd.dma_start(wkv_sb[:, 1, co:co + 4],
                                            wkvr[:, 1, co:co + 4])
                if t >= VLAG:
                    v_chunk(t - VLAG)
                # transposes lag two iterations: their rope chains (DVE) are
                # certainly drained, so they never block the PE stream
                flush_trs(upto=t - 2)
                if t == TOKCH - 1:
                    # prewarm the exp act-table; the 1.28us load runs behind
                    # the V tail
                    nc.scalar.activation(warm, zero_col[0:1, :], AF.Exp)

            # lagged V tail: pure PE work that covers the final rope chains
            for tv in range(TOKCH - VLAG, TOKCH):
                v_chunk(tv)
                if tv == TOKCH - 2:
                    flush_trs()

        # ================= phase B: attention ============================
        # wp prefetch: issue at phase-B start so the tiles are resident long
        # before phase C begins (phase-A pools have closed, SBUF is free)
        wpool = ctx.enter_context(tc.tile_pool(name="wp", bufs=1))
        wpr = wp8.rearrange("ct (hc p) two c -> p ct hc two c", p=P)
        wp_ts = []
        for ct in range(C // 512):
            wp_t = wpool.tile([P, HG, 2, 512], F8E4, tag=f"wpt{ct}")
            (nc.sync if ct % 2 == 0 else nc.scalar).dma_start(
                wp_t, wpr[:, ct])
            wp_ts.append(wp_t)

        if "B" not in PHASES:
            pass
        else:
         with tc.tile_pool(name="psc", bufs=2, space="PSUM") as psc, \
             tc.tile_pool(name="psy", bufs=2, space="PSUM") as psy, \
             tc.tile_pool(name="pss", bufs=1, space="PSUM") as pss, \
             tc.tile_pool(name="po", bufs=1, space="PSUM") as pso, \
             tc.tile_pool(name="pb", bufs=8) as ppool, \
             tc.tile_pool(name="sb", bufs=4) as bpool, \
             tc.tile_pool(name="so", bufs=3) as opool:
            NT = T // 512  # 4 tq tiles
            OSC = 1.0 / (YS * WS)
            cq = []  # pending output-projection tiles, emitted at head
            #          boundaries to fill the PE while psum chains drain
            ci = 0

            def emit_c(tc_, ct, last=False, alt=False):
                nonlocal ci
                wp_t = wp_ts[ct]
                # in the tail (alt=True) the psy ring is idle: alternate
                # between the pso bank and psy's so back-to-back projection
                # tiles don't serialize on one psum evacuation
                if alt and ci % 2 == 0:
                    ps_o = psy.tile([P, 512], F32, tag="y")
                else:
                    ps_o = pso.tile([P, 512], F32, tag="o")
                ob = opool.tile([P, 512], F32, tag="ob")
                tsl = slice(tc_ * P, (tc_ + 1) * P)

                def proj(cs, ps):
                    for pi, (a, bi) in enumerate(
                            ((y8h, 0), (y8h, 1), (y8l, 0))):
                        for hc in range(0, HG, 2):
                            nc.tensor.matmul(
                                ps, a[:, hc:hc + 2, tsl],
                                wp_t[:, hc:hc + 2, bi, cs],
                                start=(pi == 0 and hc == 0),
                                stop=(pi == 2 and hc == HG - 2),
                                perf_mode=DR)

                if not last:
                    proj(slice(0, 512), ps_o)
                    ci += 1
                    if ci % 2 == 0:
                        nc.vector.tensor_scalar_mul(ob, ps_o, OSC)
                    else:
                        nc.scalar.mul(ob, ps_o, OSC)
                    (nc.sync if ci % 2 == 0 else nc.scalar).dma_start(
                        out[tc_ * P:(tc_ + 1) * P, ct * 512:(ct + 1) * 512], ob)
                else:
                    # final tile in two pipelined halves to shorten the
                    # copy->dma drain tail
                    for q in range(2):
                        cs = slice(q * 256, (q + 1) * 256)
                        proj(cs, ps_o[:, cs])
                        if q == 0:
                            nc.vector.tensor_scalar_mul(ob[:, cs],
                                                        ps_o[:, cs], OSC)
                        else:
                            nc.scalar.mul(ob[:, cs], ps_o[:, cs], OSC)
                        (nc.sync if q == 0 else nc.scalar).dma_start(
                            out[tc_ * P:(tc_ + 1) * P,
                                ct * 512 + q * 256:ct * 512 + (q + 1) * 256],
                            ob[:, cs])
            # software pipeline over chunk PAIRS: scores+exp for pair
            # idx+DEPTH are emitted before pv of pair idx, so the
            # scores->mask->exp chain hides behind PE work.
            DEPTH = 3
            # tile order: start with a mid-length tile so the first tile's
            # head boundaries are not too short, then feed each finished
            # tile's projection tiles into the next tile's head boundaries
            # (the short-head t=0/1 tiles get C-fill this way too)
            TORD = [0, 1, 2, 3]
            for ti, t in enumerate(TORD):
                nch = 4 * (t + 1)
                npair = nch // 2
                items = [(h, pr) for h in range(HG) for pr in range(npair)]
                live = {}
                # previously finished tq-tile's projection tiles
                if ti >= 1:
                    tprev = TORD[ti - 1]
                    cq.extend((tc_, ct) for tc_ in range(4 * tprev,
                                                         4 * tprev + 4)
                              for ct in range(C // 512))

                def front(idx):
                    h, pr = items[idx]
                    g = h // NREP
                    c0 = 2 * pr
                    ps_sc = psc.tile([P, 2, 512], F32, tag="sc")
                    pt = ppool.tile([P, 2, 512], F16, tag="pt")
                    col0s = []
                    for i, c in enumerate((c0, c0 + 1)):
                        o = c * P - t * 512
                        col0 = max(o, 0)
                        col0s.append(col0)
                        nc.tensor.matmul(
                            ps_sc[:, i, col0:512], kT[:, g, c * P:(c + 1) * P],
                            qT[:, h, t * 512 + col0:(t + 1) * 512],
                            start=True, stop=(o < 0))
                        if o >= 0:
                            # after the col0 shift the partial block is always
                            # the i' >= j triangle; accumulate the additive
                            # mask with a 128-col matmul right behind scores
                            nc.tensor.matmul(ps_sc[:, i, col0:col0 + P], ident,
                                             mask_sb, start=False, stop=True)
                    a0, a1 = col0s
                    # one exp instruction across both psum banks; for a
                    # diagonal pair the flat range [a0:1024] includes the
                    # stale segment [512:512+a1) -- exp of an old (finite)
                    # score lands in a pt region that nothing reads
                    nc.scalar.activation(
                        pt.rearrange("p a b -> p (a b)")[:, a0:1024],
                        ps_sc.rearrange("p a b -> p (a b)")[:, a0:1024],
                        AF.Exp, bias=EXPB)
                    # fp16 pair-sum for the denominator tree (DVE 2x); for
                    # the diagonal pairs only the region >= a0 is live, and
                    # [a0, a1) has just the first chunk
                    s01 = bpool.tile([P, 512], F16, tag="s01")
                    if a1 > a0:
                        nc.vector.tensor_copy(s01[:, a0:a1], pt[:, 0, a0:a1])
                        nc.vector.tensor_add(s01[:, a1:512], pt[:, 0, a1:512],
                                             pt[:, 1, a1:512])
                    else:
                        nc.vector.tensor_add(s01, pt[:, 0, :], pt[:, 1, :])
                    live[idx] = (pt, col0s, s01)

                for i in range(min(DEPTH, len(items))):
                    front(i)
                ys = {}
                s01s = {}
                for idx, (h, pr) in enumerate(items):
                    if idx + DEPTH < len(items):
                        front(idx + DEPTH)
                    g = h // NREP
                    c0 = 2 * pr
                    if pr == 0:
                        ps_y = psy.tile([P, 512], F32, tag="y")
                        ps_s = pss.tile([P, 512], F32, tag="s")
                        ys[h] = (ps_y, ps_s)
                    ps_y, ps_s = ys[h]
                    pt, col0s, s01 = live.pop(idx)
                    for i, c in enumerate((c0, c0 + 1)):
                        col0 = col0s[i]
                        nc.tensor.matmul(ps_y[:, col0:512],
                                         v_sb[:, c, g * P:(g + 1) * P],
                                         pt[:, i, col0:512],
                                         start=(pr == 0 and i == 0),
                                         stop=(pr == npair - 1 and i == 1))
                    # denominator tree: full pairs combine to quads on the
                    # Pool engine (one ones-matmul per quad); the two
                    # diagonal pairs feed the ones-matmul directly
                    nq = npair - 2  # full pairs; always even
                    if pr < nq:
                        if pr % 2 == 0:
                            s01s[h] = s01
                        else:
                            q4 = bpool.tile([P, 512], F16, tag="q4")
                            nc.vector.tensor_tensor(q4, s01s.pop(h), s01,
                                                    ALU.add)
                            nc.tensor.matmul(ps_s, ones_mat, q4,
                                             start=(pr == 1),
                                             stop=False)
                    else:
                        a0 = col0s[0]
                        nc.tensor.matmul(ps_s[:, a0:512], ones_mat,
                                         s01[:, a0:512],
                                         start=(pr == nq and nq == 0),
                                         stop=(pr == npair - 1))
                    if pr == npair - 1:
                        # normalize + split into the fp8 hi/lo
                        # output-projection operand (den is already
                        # replicated across partitions by ones_mat)
                        rc = bpool.tile([P, 512], F32, tag="rc")
                        nc.vector.reciprocal(rc, ps_s)
                        yn = bpool.tile([P, 512], BF16, tag="yn")
                        nc.vector.scalar_tensor_tensor(
                            yn, ps_y, YS, rc, op0=ALU.mult, op1=ALU.mult)
                        yh = y8h[:, h, t * 512:(t + 1) * 512]
                        nc.vector.tensor_copy(yh, yn)
                        nc.vector.scalar_tensor_tensor(
                            y8l[:, h, t * 512:(t + 1) * 512], yn, 1.0, yh,
                            op0=ALU.mult, op1=ALU.subtract)
                        # fill the head-boundary psum-chain drain with two
                        # output-projection tiles of the previous tq-tile
                        for _ in range(2):
                            if cq:
                                tc_, ct = cq.pop(0)
                                emit_c(tc_, ct)

            # tail: the final processed tq-tile's projection tiles
            while cq:
                tc_, ct = cq.pop(0)
                emit_c(tc_, ct, alt=True)
            tlast = TORD[-1]
            ctail = [(tc_, ct) for tc_ in range(4 * tlast, 4 * tlast + 4)
                     for ct in range(C // 512)]
            for tc_, ct in ctail:
                emit_c(tc_, ct, alt=True,
                       last=((tc_, ct) == ctail[-1]))

    nc.compile()
    return nc


_NC_CACHE = []


def _get_prog():
    if not _NC_CACHE:
        _NC_CACHE.append(_build())
    return _NC_CACHE[0]


def _split8(a, axis):
    """Stack (hi, lo) fp8 split along a new axis."""
    e4 = ml_dtypes.float8_e4m3
    hi = a.astype(e4)
    lo = (a - hi.astype(np.float32)).astype(e4)
    return np.ascontiguousarray(np.stack([hi, lo], axis=axis))


def _make_in_maps(inputs):
    x, cos, sin = inputs["x"], inputs["cos"], inputs["sin"]
    wq, wk, wv, wproj = inputs["wq"], inputs["wk"], inputs["wv"], inputs["wproj"]
    bf = ml_dtypes.bfloat16
    # [p, tc, d] tiling (contiguous DMA rows)
    cos2 = np.ascontiguousarray(
        cos.reshape(TOKCH, P, D // 2).transpose(1, 0, 2)).astype(bf)
    sin2 = np.ascontiguousarray(
        sin.reshape(TOKCH, P, D // 2).transpose(1, 0, 2)).astype(bf)
    in_maps = []
    for core in range(8):
        b, g = core // 2, core % 2
        qs = slice(g * QC, (g + 1) * QC)
        ks = slice(g * KC, (g + 1) * KC)
        # x[b].T is [C, T]; tile to [tokch, p(C-chunk), co, tk]
        xtb = (x[b].T.astype(np.float32)
               .reshape(C // P, P, TOKCH, P)     # [co, p, tc, tk]
               .transpose(2, 1, 0, 3))           # [tc, p, co, tk]
        wq2 = _split8(np.ascontiguousarray(wq[:, qs]) * WS, axis=1)
        wkv2 = _split8(np.hstack([wk[:, ks], wv[:, ks]]) * WS, axis=1)
        wp2 = _split8(np.ascontiguousarray(wproj[qs, :]) * WS, axis=1)
        in_maps.append({
            "xt8": _split8(xtb, axis=2),                 # [tc, p, 2, co, tk]
            # [ch, C, 2, 512] / [kv, C, 2, KC] / [ct, QC, 2, 512]
            "wq8": np.ascontiguousarray(
                wq2.reshape(C, 2, 2, 512).transpose(2, 0, 1, 3)),
            "wkv8": np.ascontiguousarray(
                wkv2.reshape(C, 2, 2, KC).transpose(2, 0, 1, 3)),
            "wp8": np.ascontiguousarray(
                wp2.reshape(QC, 2, 4, 512).transpose(2, 0, 1, 3)),
            "cosd": cos2,
            "sind": sin2,
        })
    return in_maps


def kernel(x, cos, sin, wq, wk, wv, wproj):
    nc = _get_prog()
    in_maps = _make_in_maps(dict(x=x, cos=cos, sin=sin, wq=wq, wk=wk, wv=wv, wproj=wproj))
    res = run_bass_kernel_spmd(nc, in_maps, core_ids=list(range(8))).results
    outp = np.empty((B, T, C), np.float32)
    for b in range(B):
        outp[b] = res[2 * b]["out"] + res[2 * b + 1]["out"]
    return outp


# revision 78
# speedup vs baseline: 1.2388x; 1.0107x over previous
"""Causal self-attention (GQA + RoPE + QK-norm) Trainium2 Bass kernel.

Sharding: 8 cores = 4 batches x 2 head-groups.  Core c -> batch c//2,
q heads (c%2)*8..+8, kv heads (c%2)*2..+2.  wproj is row-sharded, so each
core emits a partial (T, C) output; the host sums the two partials per batch.

Device-side layout strategy (per core):
  - x and the projection weights ship as error-compensated fp8 hi/lo pairs
    (hi = fp8(v), lo = fp8(v - hi)); the QKV and output projections run as
    3-pass DoubleRow fp8 matmuls (hi*hi + hi*lo + lo*hi, dropping the
    ~0.4%-scale lo*lo term) -- 4/3x faster than bf16 on the PE.
  - QKV projections produce Q,K token-major.  rms-norm runs BEFORE rope
    (rope is a rotation per frequency pair, so it preserves the per-head
    norm): the psum evacuation applies the per-head 1/rms as the Act-copy
    scale, then the rope runs as an all-bf16 DVE chain (2x mode), and the
    128x128 PE transposes produce qT/kT feature-major.  V is token-major
    fp16, which is exactly the p@v stationary layout.
  - scores are computed transposed (scoresT[tk, tq]) in bf16, psums in
    2-chunk pairs so one exp covers 1024 columns; exp applies bias -4 and
    writes fp16 p-tiles (fits fp16 range since qk-norm bounds |s|<=11.32;
    the bias cancels between numerator and denominator).
  - softmax denominator: fp16 pair-sums on the DVE + quad-sums on the Pool
    engine feed a ones-column matmul per quad -- the PE streams den at 1/4
    of the direct cost.
  - output projection: yT is normalized and split into fp8 hi/lo during
    phase B; 3-pass fp8 DoubleRow over hc pairs.  Partial written fp32.
"""

import numpy as np
import ml_dtypes
from contextlib import ExitStack

import concourse.bass as bass
import concourse.mybir as mybir
import concourse.tile as tile
from concourse import bacc
from concourse.bass_utils import run_bass_kernel_spmd
from concourse.masks import make_identity

BF16 = mybir.dt.bfloat16
F16 = mybir.dt.float16
F32 = mybir.dt.float32
F8E4 = mybir.dt.float8e4
DR = mybir.MatmulPerfMode.DoubleRow
AF = mybir.ActivationFunctionType
ALU = mybir.AluOpType

B, T, C = 4, 2048, 2048
H, KV, D = 16, 4, 128
HG, KVG = H // 2, KV // 2          # per-core q heads (8), kv heads (2)
QC, KC = HG * D, KVG * D           # 1024, 256
P = 128
TOKCH = T // P                     # 16 token chunks
NREP = H // KV                     # 4
EPS = 1e-5
NEG = -1.0e5                       # additive causal mask (exp -> 0)
WS = 32.0                          # fp8 weight pre-scale (host): keeps w
                                   # entries in e4m3 normal range; q/k are
                                   # scale-invariant through rms-norm, v is
                                   # rescaled by 1/WS at psum evacuation
YS = 8.0                           # y pre-scale for the fp8 hi/lo split of
                                   # the output-projection LHS
EXPB = -4.0                        # exp(s-4) so p fits fp16 range; cancels
                                   # between numerator and denominator


DEBUG_DUMP = False
PHASES = ("A", "B", "C")


def _build():
    nc = bacc.Bacc("TRN2", target_bir_lowering=False, debug=False, num_devices=8)
    # x pre-tiled by the host as [tokch, p, co, tk] so every DMA partition row
    # is contiguous; hi/lo fp8 pair (combined traffic = bf16 baseline)
    # hi/lo fp8 pairs ship interleaved in one tensor per operand: one DMA
    # carries both (the HWDGE issue sequencer at ~625ns/DMA is the startup
    # bottleneck, so DMA count matters more than size)
    xt8 = nc.dram_tensor("xt8", [TOKCH, P, 2, C // P, P], F8E4,
                         kind="ExternalInput")
    # outer dims split by first use so every DMA slice stays <=3 dims:
    # wq8 by column half, wkv8 by K/V, wp8 by ct chunk
    wq8 = nc.dram_tensor("wq8", [2, C, 2, 512], F8E4, kind="ExternalInput")
    wkv8 = nc.dram_tensor("wkv8", [2, C, 2, KC], F8E4, kind="ExternalInput")
    wp8 = nc.dram_tensor("wp8", [4, QC, 2, 512], F8E4, kind="ExternalInput")
    # cos/sin pre-tiled by host as [p, tc, d] bf16 (keeps the rope chain in
    # the DVE 2x mode)
    cosd = nc.dram_tensor("cosd", [P, TOKCH, D // 2], BF16, kind="ExternalInput")
    sind = nc.dram_tensor("sind", [P, TOKCH, D // 2], BF16, kind="ExternalInput")
    out = nc.dram_tensor("out", [T, C], F32, kind="ExternalOutput")

    with tile.TileContext(nc) as tc, ExitStack() as ctx:
        singles = ctx.enter_context(tc.tile_pool(name="singles", bufs=1))
        # bufs must cover the V-lag window (xtile(t) is re-read by the lagged
        # V projection at iteration t+VLAG); the pool closes with phase A
        phase_a_pools = ExitStack()
        xpool = phase_a_pools.enter_context(tc.tile_pool(name="xa", bufs=8))

        # ---- resident tensors ----
        wq_sb = singles.tile([P, 2, C // P, 2, 512], F8E4)
        wkv_sb = singles.tile([P, 2, C // P, 2, KC], F8E4)
        wqr = wq8.rearrange("ch (co p) two q -> p ch co two q", p=P)
        wkvr = wkv8.rearrange("kv (co p) two q -> p kv co two q", p=P)
        cos_sb = singles.tile([P, TOKCH, D // 2], BF16)
        sin_sb = singles.tile([P, TOKCH, D // 2], BF16)
        # x hi/lo tiles for iteration 0 go out first (they gate the PE),
        # then weights in consumption order (first q pair, k halves, q bulk,
        # V last -- first consumed at iteration VLAG)
        xtile0 = xpool.tile([P, 2, C // P, P], F8E4, tag="xt")
        nc.sync.dma_start(xtile0[:, :, 0:4, :], xt8[0, :, :, 0:4, :])
        # weight DMAs in exact consumption order (co-major, q columns
        # before k); hi+lo ride together so the issue count stays low
        qs2 = [nc.sync, nc.scalar]
        for i, co in enumerate(range(0, C // P, 4)):
            qs2[i % 2].dma_start(wq_sb[:, 0, co:co + 4], wqr[:, 0, co:co + 4])
        nc.scalar.dma_start(xtile0[:, :, 4:16, :], xt8[0, :, :, 4:16, :])
        for i, co in enumerate(range(0, C // P, 8)):
            qs2[i % 2].dma_start(wq_sb[:, 1, co:co + 8], wqr[:, 1, co:co + 8])
        for i, co in enumerate(range(0, C // P, 8)):
            qs2[(i + 1) % 2].dma_start(wkv_sb[:, 0, co:co + 8],
                                       wkvr[:, 0, co:co + 8])
        # x tiles for t=1,2 ahead of the V-weight bulk
        xtile_pre = {}
        for tp in (1, 2):
            xp = xpool.tile([P, 2, C // P, P], F8E4, tag="xt")
            qs2[tp % 2].dma_start(xp, xt8[tp])
            xtile_pre[tp] = xp
        nc.gpsimd.dma_start(cos_sb, cosd[:])
        nc.gpsimd.dma_start(sin_sb, sind[:])

        ident = singles.tile([P, P], BF16)
        make_identity(nc, ident)
        # ones MATRIX: stationary for the denominator matmuls -> the psum
        # output carries the tk-sum replicated across all 128 partitions
        # (same cost as a 1-row output; kills the partition_broadcast)
        ones_mat = singles.tile([P, P], F16)
        nc.vector.memset(ones_mat, 1.0)
        zero_col = singles.tile([P, 1], F32)
        nc.vector.memset(zero_col, 0.0)
        eps_col = singles.tile([P, 1], F32)
        nc.vector.memset(eps_col, EPS)
        nb_col = singles.tile([P, 1], F32)
        nc.vector.memset(nb_col, EXPB)
        nc.const_aps.aps[(F32, 0.0)] = zero_col[:]
        nc.const_aps.aps[(F32, EPS)] = eps_col[:]
        nc.const_aps.aps[(F32, EXPB)] = nb_col[:]
        # scratch for the dummy exp that prewarms the exp act-table at the
        # A->B phase boundary (overlaps the 1.28us table load)
        warm = singles.tile([1, 1], F32)

        # diagonal-block mask: keep where i >= j (j = tk partition, i = tq
        # free).  bf16 so it can be ADDED into the scores psum by a 128-col
        # matmul (ident.T @ mask) instead of a DVE op in the exp chain.
        mask_sb = singles.tile([P, P], BF16)
        nc.vector.memset(mask_sb, 0.0)
        nc.gpsimd.affine_select(
            out=mask_sb, in_=mask_sb,
            compare_op=ALU.is_ge, fill=NEG,
            base=0, pattern=[[1, P]], channel_multiplier=-1,
        )

        qT = singles.tile([P, HG, T], BF16)      # [d, h, tok]
        kT = singles.tile([P, KVG, T], BF16)
        v_sb = singles.tile([P, TOKCH, KC], F16)  # [tok%128, chunk, vcol]
        # output-projection LHS, fp8 hi/lo split of yT*YS (written in B)
        y8h = singles.tile([P, HG, T], F8E4)
        y8l = singles.tile([P, HG, T], F8E4)

        # ================= phase A: QKV proj + norm + RoPE ================
        if "A" not in PHASES:
            pass
        else:
         with phase_a_pools, \
             tc.tile_pool(name="pa", bufs=2, space="PSUM") as pps, \
             tc.tile_pool(name="pkv", bufs=1, space="PSUM") as pkv, \
             tc.tile_pool(name="sa", bufs=3) as spool:
            VLAG = 6
            nco = C // P
            xtiles = {}
            # transposes lag one iteration behind their rope chain so they
            # never sit dep-blocked in the PE's 4-deep wait queue
            pending_tr = []

            def flush_trs(upto=None):
                while pending_tr and (upto is None or pending_tr[0][4] <= upto):
                    qbf, dstT, h0, nh, tt = pending_tr.pop(0)
                    pst = pkv.tile([P, 4, P], BF16, tag="tr")
                    for i in range(nh):
                        nc.tensor.transpose(pst[:, i, :], qbf[:, i, :], ident)
                    nc.scalar.copy(
                        dstT[:, h0:h0 + nh, tt * P:(tt + 1) * P], pst[:, 0:nh, :])

            def mm3(ps, xt, w_sb, half):
                # 3-pass hi/lo fp8 DoubleRow accumulation, co-major so the
                # weight consumption follows the startup DMA stream order
                for co in range(0, nco, 2):
                    for pi, (ai, bi) in enumerate(((0, 0), (0, 1), (1, 0))):
                        nc.tensor.matmul(
                            ps, xt[:, ai, co:co + 2, :],
                            w_sb[:, half, co:co + 2, bi, :],
                            start=(pi == 0 and co == 0),
                            stop=(pi == 2 and co == nco - 2),
                            perf_mode=DR)

            def v_chunk(tv):
                xv = xtiles.pop(tv)
                ps_v = pkv.tile([P, KC], F32, tag="v")
                mm3(ps_v, xv, wkv_sb, 1)
                # cast straight to resident token-major fp16 buffer; 1/WS
                # undoes the host-side fp8 weight pre-scale
                nc.scalar.mul(v_sb[:, tv, :], ps_v, 1.0 / WS)

            for t in range(TOKCH):
                if t == 0:
                    xtile = xtile0
                elif t in xtile_pre:
                    xtile = xtile_pre[t]
                else:
                    xtile = xpool.tile([P, 2, C // P, P], F8E4, tag="xt")
                    (nc.sync if t % 2 == 0 else nc.scalar).dma_start(
                        xtile, xt8[t])
                xtiles[t] = xtile
                ps_q0 = pps.tile([P, 512], F32, tag="q0")
                ps_q1 = pps.tile([P, 512], F32, tag="q1")
                ps_k = pkv.tile([P, KC], F32, tag=("k" if t % 2 == 0 else "k2"))

                # rms-norm stats run off the psum (pre-rope: rope is a
                # rotation per frequency pair so it preserves per-head
                # norms); the rope chain itself runs on an all-bf16
                # evacuation (DVE 2x mode), and the 1/rms multiplier is
                # applied once at the end.
                def norm_rope(ps, nh, dstT, h0, qscale):
                    h2 = D // 2
                    psv = ps.rearrange("p (h d) -> p h d", h=nh)
                    sq = spool.tile([P, nh, D], BF16, tag=f"sq{h0}_{nh}")
                    nc.scalar.activation(sq, psv, AF.Square)
                    ss = spool.tile([P, nh], BF16, tag=f"ss{h0}_{nh}")
                    with nc.allow_low_precision(
                            reason="rms stats; 0.4% scale err is fine"):
                        nc.vector.tensor_reduce(ss, sq,
                                                axis=mybir.AxisListType.X,
                                                op=ALU.add)
                    rt = spool.tile([P, nh], F32, tag=f"rt{h0}_{nh}")
                    nc.scalar.activation(rt, ss, AF.Sqrt, scale=1.0 / D,
                                         bias=EPS)
                    rq = spool.tile([P, nh], F32, tag=f"rq{h0}_{nh}")
                    nc.vector.reciprocal(rq, rt)
                    if qscale != 1.0:
                        nc.vector.tensor_scalar_mul(rq, rq, qscale)
                    qe = spool.tile([P, nh, D], BF16, tag=f"qe{h0}_{nh}")
                    nc.scalar.copy(qe, psv)
                    # rope on the bf16 evacuation (all-2-byte DVE ops)
                    q1, q2 = qe[:, :, 0:h2], qe[:, :, h2:D]
                    r = spool.tile([P, nh, 2, h2], BF16, tag=f"ro{h0}_{nh}")
                    r1, r2 = r[:, :, 0, :], r[:, :, 1, :]
                    s2 = spool.tile([P, nh, h2], BF16, tag=f"sc{h0}_{nh}")
                    cs = cos_sb[:, t, None, :].to_broadcast([P, nh, h2])
                    sn = sin_sb[:, t, None, :].to_broadcast([P, nh, h2])
                    nc.vector.tensor_mul(r1, q1, cs)
                    nc.vector.tensor_mul(s2, q2, sn)
                    nc.vector.tensor_sub(r1, r1, s2)
                    nc.vector.tensor_mul(r2, q1, sn)
                    nc.vector.tensor_mul(s2, q2, cs)
                    nc.vector.tensor_add(r2, r2, s2)
                    rf = r.rearrange("p h a d -> p h (a d)")
                    qbf = spool.tile([P, nh, D], BF16, tag=f"qb{h0}_{nh}")
                    nc.vector.tensor_mul(
                        qbf, rf, rq[:, :, None].to_broadcast([P, nh, D]))
                    pending_tr.append((qbf, dstT, h0, nh, t))

                qsc = 1.0 / float(np.sqrt(D))
                mm3(ps_q0, xtile, wq_sb, 0)
                mm3(ps_q1, xtile, wq_sb, 1)
                mm3(ps_k, xtile, wkv_sb, 0)
                norm_rope(ps_q0, 4, qT, 0, qsc)
                norm_rope(ps_q1, 4, qT, 4, qsc)
                norm_rope(ps_k, KVG, kT, 0, 1.0)
                if t == 2:
                    # V weights: first consumed at t=VLAG; issuing here keeps
                    # their transfers out of the startup-critical DMA window
                    for co in range(0, C // P, 4):
                        nc.gpsimd.dma_start(wkv_sb[:, 1, co:co + 4],
                                            wkvr[:, 1, co:co + 4])
                if t >= VLAG:
                    v_chunk(t - VLAG)
                # transposes lag two iterations: their rope chains (DVE) are
                # certainly drained, so they never block the PE stream
                flush_trs(upto=t - 2)
                if t == TOKCH - 1:
                    # prewarm the exp act-table; the 1.28us load runs behind
                    # the V tail
                    nc.scalar.activation(warm, zero_col[0:1, :], AF.Exp)

            # lagged V tail: pure PE work that covers the final rope chains
            for tv in range(TOKCH - VLAG, TOKCH):
                v_chunk(tv)
                if tv == TOKCH - 2:
                    flush_trs()

        # ================= phase B: attention ============================
        # wp prefetch: issue at phase-B start so the tiles are resident long
        # before phase C begins (phase-A pools have closed, SBUF is free)
        wpool = ctx.enter_context(tc.tile_pool(name="wp", bufs=1))
        wpr = wp8.rearrange("ct (hc p) two c -> p ct hc two c", p=P)
        wp_ts = []
        for ct in range(C // 512):
            wp_t = wpool.tile([P, HG, 2, 512], F8E4, tag=f"wpt{ct}")
            (nc.sync if ct % 2 == 0 else nc.scalar).dma_start(
                wp_t, wpr[:, ct])
            wp_ts.append(wp_t)

        if "B" not in PHASES:
            pass
        else:
         with tc.tile_pool(name="psc", bufs=2, space="PSUM") as psc, \
             tc.tile_pool(name="psy", bufs=2, space="PSUM") as psy, \
             tc.tile_pool(name="pss", bufs=1, space="PSUM") as pss, \
             tc.tile_pool(name="po", bufs=1, space="PSUM") as pso, \
             tc.tile_pool(name="pb", bufs=8) as ppool, \
             tc.tile_pool(name="sb", bufs=4) as bpool, \
             tc.tile_pool(name="s1", bufs=6) as s1pool, \
             tc.tile_pool(name="sr", bufs=2) as rpool, \
             tc.tile_pool(name="so", bufs=3) as opool:
            NT = T // 512  # 4 tq tiles
            OSC = 1.0 / (YS * WS)
            cq = []  # pending output-projection tiles, emitted at head
            #          boundaries to fill the PE while psum chains drain
            ci = 0

            def emit_c(tc_, ct, last=False, alt=False):
                nonlocal ci
                wp_t = wp_ts[ct]
                # in the tail (alt=True) the psy ring is idle: alternate
                # between the pso bank and psy's so back-to-back projection
                # tiles don't serialize on one psum evacuation
                if alt and ci % 2 == 0:
                    ps_o = psy.tile([P, 512], F32, tag="y")
                else:
                    ps_o = pso.tile([P, 512], F32, tag="o")
                ob = opool.tile([P, 512], F32, tag="ob")
                tsl = slice(tc_ * P, (tc_ + 1) * P)

                def proj(cs, ps):
                    for pi, (a, bi) in enumerate(
                            ((y8h, 0), (y8h, 1), (y8l, 0))):
                        for hc in range(0, HG, 2):
                            nc.tensor.matmul(
                                ps, a[:, hc:hc + 2, tsl],
                                wp_t[:, hc:hc + 2, bi, cs],
                                start=(pi == 0 and hc == 0),
                                stop=(pi == 2 and hc == HG - 2),
                                perf_mode=DR)

                if not last:
                    proj(slice(0, 512), ps_o)
                    ci += 1
                    if ci % 2 == 0:
                        nc.vector.tensor_scalar_mul(ob, ps_o, OSC)
                    else:
                        nc.scalar.mul(ob, ps_o, OSC)
                    (nc.sync if ci % 2 == 0 else nc.scalar).dma_start(
                        out[tc_ * P:(tc_ + 1) * P, ct * 512:(ct + 1) * 512], ob)
                else:
                    # final tile in two pipelined halves to shorten the
                    # copy->dma drain tail
                    for q in range(2):
                        cs = slice(q * 256, (q + 1) * 256)
                        proj(cs, ps_o[:, cs])
                        if q == 0:
                            nc.vector.tensor_scalar_mul(ob[:, cs],
                                                        ps_o[:, cs], OSC)
                        else:
                            nc.scalar.mul(ob[:, cs], ps_o[:, cs], OSC)
                        (nc.sync if q == 0 else nc.scalar).dma_start(
                            out[tc_ * P:(tc_ + 1) * P,
                                ct * 512 + q * 256:ct * 512 + (q + 1) * 256],
                            ob[:, cs])
            # software pipeline over chunk PAIRS: scores+exp for pair
            # idx+DEPTH are emitted before pv of pair idx, so the
            # scores->mask->exp chain hides behind PE work.
            DEPTH = 3
            # tile order: start with a mid-length tile so the first tile's
            # head boundaries are not too short, then feed each finished
            # tile's projection tiles into the next tile's head boundaries
            # (the short-head t=0/1 tiles get C-fill this way too)
            TORD = [0, 1, 2, 3]
            for ti, t in enumerate(TORD):
                nch = 4 * (t + 1)
                npair = nch // 2
                items = [(h, pr) for h in range(HG) for pr in range(npair)]
                live = {}
                # previously finished tq-tile's projection tiles
                if ti >= 1:
                    tprev = TORD[ti - 1]
                    cq.extend((tc_, ct) for tc_ in range(4 * tprev,
                                                         4 * tprev + 4)
                              for ct in range(C // 512))

                def front(idx):
                    h, pr = items[idx]
                    g = h // NREP
                    c0 = 2 * pr
                    ps_sc = psc.tile([P, 2, 512], F32, tag="sc")
                    pt = ppool.tile([P, 2, 512], F16, tag="pt")
                    col0s = []
                    for i, c in enumerate((c0, c0 + 1)):
                        o = c * P - t * 512
                        col0 = max(o, 0)
                        col0s.append(col0)
                        nc.tensor.matmul(
                            ps_sc[:, i, col0:512], kT[:, g, c * P:(c + 1) * P],
                            qT[:, h, t * 512 + col0:(t + 1) * 512],
                            start=True, stop=(o < 0))
                        if o >= 0:
                            # after the col0 shift the partial block is always
                            # the i' >= j triangle; accumulate the additive
                            # mask with a 128-col matmul right behind scores
                            nc.tensor.matmul(ps_sc[:, i, col0:col0 + P], ident,
                                             mask_sb, start=False, stop=True)
                    a0, a1 = col0s
                    # one exp instruction across both psum banks; for a
                    # diagonal pair the flat range [a0:1024] includes the
                    # stale segment [512:512+a1) -- exp of an old (finite)
                    # score lands in a pt region that nothing reads
                    nc.scalar.activation(
                        pt.rearrange("p a b -> p (a b)")[:, a0:1024],
                        ps_sc.rearrange("p a b -> p (a b)")[:, a0:1024],
                        AF.Exp, bias=EXPB)
                    # fp16 pair-sum for the denominator tree (DVE 2x); for
                    # the diagonal pairs only the region >= a0 is live, and
                    # [a0, a1) has just the first chunk
                    s01 = s1pool.tile([P, 512], F16, tag="s01")
                    if a1 > a0:
                        nc.vector.tensor_copy(s01[:, a0:a1], pt[:, 0, a0:a1])
                        nc.vector.tensor_add(s01[:, a1:512], pt[:, 0, a1:512],
                                             pt[:, 1, a1:512])
                    else:
                        nc.vector.tensor_add(s01, pt[:, 0, :], pt[:, 1, :])
                    live[idx] = (pt, col0s, s01)

                for i in range(min(DEPTH, len(items))):
                    front(i)
                ys = {}
                s01s = {}
                q4s = {}
                diaga = {}
                dfirst = {}
                for idx, (h, pr) in enumerate(items):
                    if idx + DEPTH < len(items):
                        front(idx + DEPTH)
                    g = h // NREP
                    c0 = 2 * pr
                    if pr == 0:
                        ps_y = psy.tile([P, 512], F32, tag="y")
                        ps_s = pss.tile([P, 512], F32, tag="s")
                        ys[h] = (ps_y, ps_s)
                    ps_y, ps_s = ys[h]
                    pt, col0s, s01 = live.pop(idx)
                    for i, c in enumerate((c0, c0 + 1)):
                        col0 = col0s[i]
                        nc.tensor.matmul(ps_y[:, col0:512],
                                         v_sb[:, c, g * P:(g + 1) * P],
                                         pt[:, i, col0:512],
                                         start=(pr == 0 and i == 0),
                                         stop=(pr == npair - 1 and i == 1))
                    # denominator tree: full pairs combine to quads on the
                    # Pool engine (one ones-matmul per quad); the two
                    # diagonal pairs feed the ones-matmul directly
                    nq = npair - 2  # full pairs; always even

                    def den_mm(src, stop):
                        nc.tensor.matmul(ps_s, ones_mat, src,
                                         start=dfirst.pop(h, True), stop=stop)

                    if pr < nq:
                        if pr % 2 == 0:
                            s01s[h] = s01
                        else:
                            # quad, then oct (fp16 tree on the DVE): each
                            # extra level halves the ones-matmul PE cost
                            q4 = bpool.tile([P, 512], F16, tag="q4")
                            nc.vector.tensor_tensor(q4, s01s.pop(h), s01,
                                                    ALU.add)
                            ql = q4s.setdefault(h, [])
                            ql.append(q4)
                            if len(ql) == 2:
                                o8 = bpool.tile([P, 512], F16, tag="q4")
                                nc.vector.tensor_add(o8, ql[0], ql[1])
                                ql.clear()
                                den_mm(o8, False)
                                dfirst[h] = False
                    elif pr == nq:
                        diaga[h] = s01
                    else:
                        # merge the second diagonal pair's live region into
                        # the first, then one ones-matmul covers both
                        sA = diaga.pop(h)
                        nc.vector.tensor_add(sA[:, 256:512], sA[:, 256:512],
                                             s01[:, 256:512])
                        for q4 in q4s.pop(h, []):
                            den_mm(q4, False)
                            dfirst[h] = False
                        den_mm(sA, True)
                    if pr == npair - 1:
                        # normalize + split into the fp8 hi/lo
                        # output-projection operand (den is already
                        # replicated across partitions by ones_mat)
                        rc = rpool.tile([P, 512], F32, tag="rc")
                        nc.vector.reciprocal(rc, ps_s)
                        yn = bpool.tile([P, 512], BF16, tag="yn")
                        nc.vector.scalar_tensor_tensor(
                            yn, ps_y, YS, rc, op0=ALU.mult, op1=ALU.mult)
                        yh = y8h[:, h, t * 512:(t + 1) * 512]
                        nc.vector.tensor_copy(yh, yn)
                        nc.vector.scalar_tensor_tensor(
                            y8l[:, h, t * 512:(t + 1) * 512], yn, 1.0, yh,
                            op0=ALU.mult, op1=ALU.subtract)
                        # fill the head-boundary psum-chain drain with two
                        # output-projection tiles of the previous tq-tile
                        for _ in range(2):
                            if cq:
                                tc_, ct = cq.pop(0)
                                emit_c(tc_, ct)

            # tail: the final processed tq-tile's projection tiles
            while cq:
                tc_, ct = cq.pop(0)
                emit_c(tc_, ct, alt=True)
            tlast = TORD[-1]
            ctail = [(tc_, ct) for tc_ in range(4 * tlast, 4 * tlast + 4)
                     for ct in range(C // 512)]
            for tc_, ct in ctail:
                emit_c(tc_, ct, alt=True,
                       last=((tc_, ct) == ctail[-1]))

    nc.compile()
    return nc


_NC_CACHE = []


def _get_prog():
    if not _NC_CACHE:
        _NC_CACHE.append(_build())
    return _NC_CACHE[0]


def _split8(a, axis):
    """Stack (hi, lo) fp8 split along a new axis."""
    e4 = ml_dtypes.float8_e4m3
    hi = a.astype(e4)
    lo = (a - hi.astype(np.float32)).astype(e4)
    return np.ascontiguousarray(np.stack([hi, lo], axis=axis))


def _make_in_maps(inputs):
    x, cos, sin = inputs["x"], inputs["cos"], inputs["sin"]
    wq, wk, wv, wproj = inputs["wq"], inputs["wk"], inputs["wv"], inputs["wproj"]
    bf = ml_dtypes.bfloat16
    # [p, tc, d] tiling (contiguous DMA rows)
    cos2 = np.ascontiguousarray(
        cos.reshape(TOKCH, P, D // 2).transpose(1, 0, 2)).astype(bf)
    sin2 = np.ascontiguousarray(
        sin.reshape(TOKCH, P, D // 2).transpose(1, 0, 2)).astype(bf)
    in_maps = []
    for core in range(8):
        b, g = core // 2, core % 2
        qs = slice(g * QC, (g + 1) * QC)
        ks = slice(g * KC, (g + 1) * KC)
        # x[b].T is [C, T]; tile to [tokch, p(C-chunk), co, tk]
        xtb = (x[b].T.astype(np.float32)
               .reshape(C // P, P, TOKCH, P)     # [co, p, tc, tk]
               .transpose(2, 1, 0, 3))           # [tc, p, co, tk]
        wq2 = _split8(np.ascontiguousarray(wq[:, qs]) * WS, axis=1)
        wkv2 = _split8(np.hstack([wk[:, ks], wv[:, ks]]) * WS, axis=1)
        wp2 = _split8(np.ascontiguousarray(wproj[qs, :]) * WS, axis=1)
        in_maps.append({
            "xt8": _split8(xtb, axis=2),                 # [tc, p, 2, co, tk]
            # [ch, C, 2, 512] / [kv, C, 2, KC] / [ct, QC, 2, 512]
            "wq8": np.ascontiguousarray(
                wq2.reshape(C, 2, 2, 512).transpose(2, 0, 1, 3)),
            "wkv8": np.ascontiguousarray(
                wkv2.reshape(C, 2, 2, KC).transpose(2, 0, 1, 3)),
            "wp8": np.ascontiguousarray(
                wp2.reshape(QC, 2, 4, 512).transpose(2, 0, 1, 3)),
            "cosd": cos2,
            "sind": sin2,
        })
    return in_maps


def kernel(x, cos, sin, wq, wk, wv, wproj):
    nc = _get_prog()
    in_maps = _make_in_maps(dict(x=x, cos=cos, sin=sin, wq=wq, wk=wk, wv=wv, wproj=wproj))
    res = run_bass_kernel_spmd(nc, in_maps, core_ids=list(range(8))).results
    outp = np.empty((B, T, C), np.float32)
    for b in range(B):
        outp[b] = res[2 * b]["out"] + res[2 * b + 1]["out"]
    return outp


# revision 79
# speedup vs baseline: 1.2460x; 1.0058x over previous
"""Causal self-attention (GQA + RoPE + QK-norm) Trainium2 Bass kernel.

Sharding: 8 cores = 4 batches x 2 head-groups.  Core c -> batch c//2,
q heads (c%2)*8..+8, kv heads (c%2)*2..+2.  wproj is row-sharded, so each
core emits a partial (T, C) output; the host sums the two partials per batch.

Device-side layout strategy (per core):
  - x and the projection weights ship as error-compensated fp8 hi/lo pairs
    (hi = fp8(v), lo = fp8(v - hi)); the QKV and output projections run as
    3-pass DoubleRow fp8 matmuls (hi*hi + hi*lo + lo*hi, dropping the
    ~0.4%-scale lo*lo term) -- 4/3x faster than bf16 on the PE.
  - QKV projections produce Q,K token-major.  rms-norm runs BEFORE rope
    (rope is a rotation per frequency pair, so it preserves the per-head
    norm): the psum evacuation applies the per-head 1/rms as the Act-copy
    scale, then the rope runs as an all-bf16 DVE chain (2x mode), and the
    128x128 PE transposes produce qT/kT feature-major.  V is token-major
    fp16, which is exactly the p@v stationary layout.
  - scores are computed transposed (scoresT[tk, tq]) in bf16, psums in
    2-chunk pairs so one exp covers 1024 columns; exp applies bias -4 and
    writes fp16 p-tiles (fits fp16 range since qk-norm bounds |s|<=11.32;
    the bias cancels between numerator and denominator).
  - softmax denominator: fp16 pair-sums on the DVE + quad-sums on the Pool
    engine feed a ones-column matmul per quad -- the PE streams den at 1/4
    of the direct cost.
  - output projection: yT is normalized and split into fp8 hi/lo during
    phase B; 3-pass fp8 DoubleRow over hc pairs.  Partial written fp32.
"""

import numpy as np
import ml_dtypes
from contextlib import ExitStack

import concourse.bass as bass
import concourse.mybir as mybir
import concourse.tile as tile
from concourse import bacc
from concourse.bass_utils import run_bass_kernel_spmd
from concourse.masks import make_identity

BF16 = mybir.dt.bfloat16
F16 = mybir.dt.float16
F32 = mybir.dt.float32
F8E4 = mybir.dt.float8e4
DR = mybir.MatmulPerfMode.DoubleRow
AF = mybir.ActivationFunctionType
ALU = mybir.AluOpType

B, T, C = 4, 2048, 2048
H, KV, D = 16, 4, 128
HG, KVG = H // 2, KV // 2          # per-core q heads (8), kv heads (2)
QC, KC = HG * D, KVG * D           # 1024, 256
P = 128
TOKCH = T // P                     # 16 token chunks
NREP = H // KV                     # 4
EPS = 1e-5
NEG = -30.0                        # additive causal mask: qk-norm bounds
                                   # |s|<=11.32, so exp(s+NEG-4) is <=1e-10
                                   # of any unmasked term; -30 is exact in
                                   # e4m3 so the mask-add can run as an fp8
                                   # DoubleRow matmul at half cost
WS = 32.0                          # fp8 weight pre-scale (host): keeps w
                                   # entries in e4m3 normal range; q/k are
                                   # scale-invariant through rms-norm, v is
                                   # rescaled by 1/WS at psum evacuation
YS = 8.0                           # y pre-scale for the fp8 hi/lo split of
                                   # the output-projection LHS
EXPB = -4.0                        # exp(s-4) so p fits fp16 range; cancels
                                   # between numerator and denominator


DEBUG_DUMP = False
PHASES = ("A", "B", "C")


def _build():
    nc = bacc.Bacc("TRN2", target_bir_lowering=False, debug=False, num_devices=8)
    # x pre-tiled by the host as [tokch, p, co, tk] so every DMA partition row
    # is contiguous; hi/lo fp8 pair (combined traffic = bf16 baseline)
    # hi/lo fp8 pairs ship interleaved in one tensor per operand: one DMA
    # carries both (the HWDGE issue sequencer at ~625ns/DMA is the startup
    # bottleneck, so DMA count matters more than size)
    xt8 = nc.dram_tensor("xt8", [TOKCH, P, 2, C // P, P], F8E4,
                         kind="ExternalInput")
    # outer dims split by first use so every DMA slice stays <=3 dims:
    # wq8 by column half, wkv8 by K/V, wp8 by ct chunk
    wq8 = nc.dram_tensor("wq8", [2, C, 2, 512], F8E4, kind="ExternalInput")
    wkv8 = nc.dram_tensor("wkv8", [2, C, 2, KC], F8E4, kind="ExternalInput")
    wp8 = nc.dram_tensor("wp8", [4, QC, 2, 512], F8E4, kind="ExternalInput")
    # cos/sin pre-tiled by host as [p, tc, d] bf16 (keeps the rope chain in
    # the DVE 2x mode)
    cosd = nc.dram_tensor("cosd", [P, TOKCH, D // 2], BF16, kind="ExternalInput")
    sind = nc.dram_tensor("sind", [P, TOKCH, D // 2], BF16, kind="ExternalInput")
    out = nc.dram_tensor("out", [T, C], F32, kind="ExternalOutput")

    with tile.TileContext(nc) as tc, ExitStack() as ctx:
        singles = ctx.enter_context(tc.tile_pool(name="singles", bufs=1))
        # bufs must cover the V-lag window (xtile(t) is re-read by the lagged
        # V projection at iteration t+VLAG); the pool closes with phase A
        phase_a_pools = ExitStack()
        xpool = phase_a_pools.enter_context(tc.tile_pool(name="xa", bufs=8))

        # ---- resident tensors ----
        wq_sb = singles.tile([P, 2, C // P, 2, 512], F8E4)
        wkv_sb = singles.tile([P, 2, C // P, 2, KC], F8E4)
        wqr = wq8.rearrange("ch (co p) two q -> p ch co two q", p=P)
        wkvr = wkv8.rearrange("kv (co p) two q -> p kv co two q", p=P)
        cos_sb = singles.tile([P, TOKCH, D // 2], BF16)
        sin_sb = singles.tile([P, TOKCH, D // 2], BF16)
        # x hi/lo tiles for iteration 0 go out first (they gate the PE),
        # then weights in consumption order (first q pair, k halves, q bulk,
        # V last -- first consumed at iteration VLAG)
        xtile0 = xpool.tile([P, 2, C // P, P], F8E4, tag="xt")
        nc.sync.dma_start(xtile0[:, :, 0:4, :], xt8[0, :, :, 0:4, :])
        # weight DMAs in exact consumption order (co-major, q columns
        # before k); hi+lo ride together so the issue count stays low
        qs2 = [nc.sync, nc.scalar]
        for i, co in enumerate(range(0, C // P, 4)):
            qs2[i % 2].dma_start(wq_sb[:, 0, co:co + 4], wqr[:, 0, co:co + 4])
        nc.scalar.dma_start(xtile0[:, :, 4:16, :], xt8[0, :, :, 4:16, :])
        for i, co in enumerate(range(0, C // P, 8)):
            qs2[i % 2].dma_start(wq_sb[:, 1, co:co + 8], wqr[:, 1, co:co + 8])
        for i, co in enumerate(range(0, C // P, 8)):
            qs2[(i + 1) % 2].dma_start(wkv_sb[:, 0, co:co + 8],
                                       wkvr[:, 0, co:co + 8])
        # x tiles for t=1,2 ahead of the V-weight bulk
        xtile_pre = {}
        for tp in (1, 2):
            xp = xpool.tile([P, 2, C // P, P], F8E4, tag="xt")
            qs2[tp % 2].dma_start(xp, xt8[tp])
            xtile_pre[tp] = xp
        nc.gpsimd.dma_start(cos_sb, cosd[:])
        nc.gpsimd.dma_start(sin_sb, sind[:])

        ident = singles.tile([P, P], BF16)
        make_identity(nc, ident)
        # ones MATRIX: stationary for the denominator matmuls -> the psum
        # output carries the tk-sum replicated across all 128 partitions
        # (same cost as a 1-row output; kills the partition_broadcast)
        ones_mat = singles.tile([P, P], F16)
        nc.vector.memset(ones_mat, 1.0)
        zero_col = singles.tile([P, 1], F32)
        nc.vector.memset(zero_col, 0.0)
        eps_col = singles.tile([P, 1], F32)
        nc.vector.memset(eps_col, EPS)
        nb_col = singles.tile([P, 1], F32)
        nc.vector.memset(nb_col, EXPB)
        nc.const_aps.aps[(F32, 0.0)] = zero_col[:]
        nc.const_aps.aps[(F32, EPS)] = eps_col[:]
        nc.const_aps.aps[(F32, EXPB)] = nb_col[:]
        # scratch for the dummy exp that prewarms the exp act-table at the
        # A->B phase boundary (overlaps the 1.28us table load)
        warm = singles.tile([1, 1], F32)

        # diagonal-block mask: keep where i >= j (j = tk partition, i = tq
        # free), ADDed into the scores psum by a matmul.  fp8 DoubleRow pair
        # (identity, 0) x (mask, 0) runs the add at 0.5 cyc/col.
        ident8 = singles.tile([P, 2, P], F8E4)
        nc.vector.memset(ident8, 0.0)
        nc.vector.tensor_copy(ident8[:, 0, :], ident)
        mask8 = singles.tile([P, 2, P], F8E4)
        nc.vector.memset(mask8, 0.0)
        nc.gpsimd.affine_select(
            out=mask8[:, 0, :], in_=mask8[:, 0, :],
            compare_op=ALU.is_ge, fill=NEG,
            base=0, pattern=[[1, P]], channel_multiplier=-1,
        )

        qT = singles.tile([P, HG, T], BF16)      # [d, h, tok]
        kT = singles.tile([P, KVG, T], BF16)
        v_sb = singles.tile([P, TOKCH, KC], F16)  # [tok%128, chunk, vcol]
        # output-projection LHS, fp8 hi/lo split of yT*YS (written in B)
        y8h = singles.tile([P, HG, T], F8E4)
        y8l = singles.tile([P, HG, T], F8E4)

        # ================= phase A: QKV proj + norm + RoPE ================
        if "A" not in PHASES:
            pass
        else:
         with phase_a_pools, \
             tc.tile_pool(name="pa", bufs=2, space="PSUM") as pps, \
             tc.tile_pool(name="pkv", bufs=1, space="PSUM") as pkv, \
             tc.tile_pool(name="sa", bufs=3) as spool:
            VLAG = 6
            nco = C // P
            xtiles = {}
            # transposes lag one iteration behind their rope chain so they
            # never sit dep-blocked in the PE's 4-deep wait queue
            pending_tr = []

            def flush_trs(upto=None):
                while pending_tr and (upto is None or pending_tr[0][4] <= upto):
                    qbf, dstT, h0, nh, tt = pending_tr.pop(0)
                    pst = pkv.tile([P, 4, P], BF16, tag="tr")
                    for i in range(nh):
                        nc.tensor.transpose(pst[:, i, :], qbf[:, i, :], ident)
                    nc.scalar.copy(
                        dstT[:, h0:h0 + nh, tt * P:(tt + 1) * P], pst[:, 0:nh, :])

            def mm3(ps, xt, w_sb, half):
                # 3-pass hi/lo fp8 DoubleRow accumulation, co-major so the
                # weight consumption follows the startup DMA stream order
                for co in range(0, nco, 2):
                    for pi, (ai, bi) in enumerate(((0, 0), (0, 1), (1, 0))):
                        nc.tensor.matmul(
                            ps, xt[:, ai, co:co + 2, :],
                            w_sb[:, half, co:co + 2, bi, :],
                            start=(pi == 0 and co == 0),
                            stop=(pi == 2 and co == nco - 2),
                            perf_mode=DR)

            def v_chunk(tv):
                xv = xtiles.pop(tv)
                ps_v = pkv.tile([P, KC], F32, tag="v")
                mm3(ps_v, xv, wkv_sb, 1)
                # cast straight to resident token-major fp16 buffer; 1/WS
                # undoes the host-side fp8 weight pre-scale
                nc.scalar.mul(v_sb[:, tv, :], ps_v, 1.0 / WS)

            for t in range(TOKCH):
                if t == 0:
                    xtile = xtile0
                elif t in xtile_pre:
                    xtile = xtile_pre[t]
                else:
                    xtile = xpool.tile([P, 2, C // P, P], F8E4, tag="xt")
                    (nc.sync if t % 2 == 0 else nc.scalar).dma_start(
                        xtile, xt8[t])
                xtiles[t] = xtile
                ps_q0 = pps.tile([P, 512], F32, tag="q0")
                ps_q1 = pps.tile([P, 512], F32, tag="q1")
                ps_k = pkv.tile([P, KC], F32, tag=("k" if t % 2 == 0 else "k2"))

                # rms-norm stats run off the psum (pre-rope: rope is a
                # rotation per frequency pair so it preserves per-head
                # norms); the rope chain itself runs on an all-bf16
                # evacuation (DVE 2x mode), and the 1/rms multiplier is
                # applied once at the end.
                def norm_rope(ps, nh, dstT, h0, qscale):
                    h2 = D // 2
                    psv = ps.rearrange("p (h d) -> p h d", h=nh)
                    sq = spool.tile([P, nh, D], BF16, tag=f"sq{h0}_{nh}")
                    nc.scalar.activation(sq, psv, AF.Square)
                    ss = spool.tile([P, nh], BF16, tag=f"ss{h0}_{nh}")
                    with nc.allow_low_precision(
                            reason="rms stats; 0.4% scale err is fine"):
                        nc.vector.tensor_reduce(ss, sq,
                                                axis=mybir.AxisListType.X,
                                                op=ALU.add)
                    rt = spool.tile([P, nh], F32, tag=f"rt{h0}_{nh}")
                    nc.scalar.activation(rt, ss, AF.Sqrt, scale=1.0 / D,
                                         bias=EPS)
                    rq = spool.tile([P, nh], F32, tag=f"rq{h0}_{nh}")
                    nc.vector.reciprocal(rq, rt)
                    if qscale != 1.0:
                        nc.vector.tensor_scalar_mul(rq, rq, qscale)
                    qe = spool.tile([P, nh, D], BF16, tag=f"qe{h0}_{nh}")
                    nc.scalar.copy(qe, psv)
                    # rope on the bf16 evacuation (all-2-byte DVE ops)
                    q1, q2 = qe[:, :, 0:h2], qe[:, :, h2:D]
                    r = spool.tile([P, nh, 2, h2], BF16, tag=f"ro{h0}_{nh}")
                    r1, r2 = r[:, :, 0, :], r[:, :, 1, :]
                    s2 = spool.tile([P, nh, h2], BF16, tag=f"sc{h0}_{nh}")
                    cs = cos_sb[:, t, None, :].to_broadcast([P, nh, h2])
                    sn = sin_sb[:, t, None, :].to_broadcast([P, nh, h2])
                    nc.vector.tensor_mul(r1, q1, cs)
                    nc.vector.tensor_mul(s2, q2, sn)
                    nc.vector.tensor_sub(r1, r1, s2)
                    nc.vector.tensor_mul(r2, q1, sn)
                    nc.vector.tensor_mul(s2, q2, cs)
                    nc.vector.tensor_add(r2, r2, s2)
                    rf = r.rearrange("p h a d -> p h (a d)")
                    qbf = spool.tile([P, nh, D], BF16, tag=f"qb{h0}_{nh}")
                    nc.vector.tensor_mul(
                        qbf, rf, rq[:, :, None].to_broadcast([P, nh, D]))
                    pending_tr.append((qbf, dstT, h0, nh, t))

                qsc = 1.0 / float(np.sqrt(D))
                mm3(ps_q0, xtile, wq_sb, 0)
                mm3(ps_q1, xtile, wq_sb, 1)
                mm3(ps_k, xtile, wkv_sb, 0)
                norm_rope(ps_q0, 4, qT, 0, qsc)
                norm_rope(ps_q1, 4, qT, 4, qsc)
                norm_rope(ps_k, KVG, kT, 0, 1.0)
                if t == 2:
                    # V weights: first consumed at t=VLAG; issuing here keeps
                    # their transfers out of the startup-critical DMA window
                    for co in range(0, C // P, 4):
                        nc.gpsimd.dma_start(wkv_sb[:, 1, co:co + 4],
                                            wkvr[:, 1, co:co + 4])
                if t >= VLAG:
                    v_chunk(t - VLAG)
                # transposes lag two iterations: their rope chains (DVE) are
                # certainly drained, so they never block the PE stream
                flush_trs(upto=t - 2)
                if t == TOKCH - 1:
                    # prewarm the exp act-table; the 1.28us load runs behind
                    # the V tail
                    nc.scalar.activation(warm, zero_col[0:1, :], AF.Exp)

            # lagged V tail: pure PE work that covers the final rope chains
            for tv in range(TOKCH - VLAG, TOKCH):
                v_chunk(tv)
                if tv == TOKCH - 2:
                    flush_trs()

        # ================= phase B: attention ============================
        # wp prefetch: issue at phase-B start so the tiles are resident long
        # before phase C begins (phase-A pools have closed, SBUF is free)
        wpool = ctx.enter_context(tc.tile_pool(name="wp", bufs=1))
        wpr = wp8.rearrange("ct (hc p) two c -> p ct hc two c", p=P)
        wp_ts = []
        for ct in range(C // 512):
            wp_t = wpool.tile([P, HG, 2, 512], F8E4, tag=f"wpt{ct}")
            (nc.sync if ct % 2 == 0 else nc.scalar).dma_start(
                wp_t, wpr[:, ct])
            wp_ts.append(wp_t)

        if "B" not in PHASES:
            pass
        else:
         with tc.tile_pool(name="psc", bufs=2, space="PSUM") as psc, \
             tc.tile_pool(name="psy", bufs=2, space="PSUM") as psy, \
             tc.tile_pool(name="pss", bufs=1, space="PSUM") as pss, \
             tc.tile_pool(name="po", bufs=1, space="PSUM") as pso, \
             tc.tile_pool(name="pb", bufs=8) as ppool, \
             tc.tile_pool(name="sb", bufs=4) as bpool, \
             tc.tile_pool(name="s1", bufs=6) as s1pool, \
             tc.tile_pool(name="sr", bufs=2) as rpool, \
             tc.tile_pool(name="so", bufs=3) as opool:
            NT = T // 512  # 4 tq tiles
            OSC = 1.0 / (YS * WS)
            cq = []  # pending output-projection tiles, emitted at head
            #          boundaries to fill the PE while psum chains drain
            ci = 0

            def emit_c(tc_, ct, last=False, alt=False):
                nonlocal ci
                wp_t = wp_ts[ct]
                # in the tail (alt=True) the psy ring is idle: alternate
                # between the pso bank and psy's so back-to-back projection
                # tiles don't serialize on one psum evacuation
                if alt and ci % 2 == 0:
                    ps_o = psy.tile([P, 512], F32, tag="y")
                else:
                    ps_o = pso.tile([P, 512], F32, tag="o")
                ob = opool.tile([P, 512], F32, tag="ob")
                tsl = slice(tc_ * P, (tc_ + 1) * P)

                def proj(cs, ps):
                    for pi, (a, bi) in enumerate(
                            ((y8h, 0), (y8h, 1), (y8l, 0))):
                        for hc in range(0, HG, 2):
                            nc.tensor.matmul(
                                ps, a[:, hc:hc + 2, tsl],
                                wp_t[:, hc:hc + 2, bi, cs],
                                start=(pi == 0 and hc == 0),
                                stop=(pi == 2 and hc == HG - 2),
                                perf_mode=DR)

                if not last:
                    proj(slice(0, 512), ps_o)
                    ci += 1
                    if ci % 2 == 0:
                        nc.vector.tensor_scalar_mul(ob, ps_o, OSC)
                    else:
                        nc.scalar.mul(ob, ps_o, OSC)
                    (nc.sync if ci % 2 == 0 else nc.scalar).dma_start(
                        out[tc_ * P:(tc_ + 1) * P, ct * 512:(ct + 1) * 512], ob)
                else:
                    # final tile in two pipelined halves to shorten the
                    # copy->dma drain tail
                    for q in range(2):
                        cs = slice(q * 256, (q + 1) * 256)
                        proj(cs, ps_o[:, cs])
                        if q == 0:
                            nc.vector.tensor_scalar_mul(ob[:, cs],
                                                        ps_o[:, cs], OSC)
                        else:
                            nc.scalar.mul(ob[:, cs], ps_o[:, cs], OSC)
                        (nc.sync if q == 0 else nc.scalar).dma_start(
                            out[tc_ * P:(tc_ + 1) * P,
                                ct * 512 + q * 256:ct * 512 + (q + 1) * 256],
                            ob[:, cs])
            # software pipeline over chunk PAIRS: scores+exp for pair
            # idx+DEPTH are emitted before pv of pair idx, so the
            # scores->mask->exp chain hides behind PE work.
            DEPTH = 3
            # tile order: start with a mid-length tile so the first tile's
            # head boundaries are not too short, then feed each finished
            # tile's projection tiles into the next tile's head boundaries
            # (the short-head t=0/1 tiles get C-fill this way too)
            TORD = [0, 1, 2, 3]
            for ti, t in enumerate(TORD):
                nch = 4 * (t + 1)
                npair = nch // 2
                items = [(h, pr) for h in range(HG) for pr in range(npair)]
                live = {}
                # previously finished tq-tile's projection tiles
                if ti >= 1:
                    tprev = TORD[ti - 1]
                    cq.extend((tc_, ct) for tc_ in range(4 * tprev,
                                                         4 * tprev + 4)
                              for ct in range(C // 512))

                def front(idx):
                    h, pr = items[idx]
                    g = h // NREP
                    c0 = 2 * pr
                    ps_sc = psc.tile([P, 2, 512], F32, tag="sc")
                    pt = ppool.tile([P, 2, 512], F16, tag="pt")
                    col0s = []
                    for i, c in enumerate((c0, c0 + 1)):
                        o = c * P - t * 512
                        col0 = max(o, 0)
                        col0s.append(col0)
                        nc.tensor.matmul(
                            ps_sc[:, i, col0:512], kT[:, g, c * P:(c + 1) * P],
                            qT[:, h, t * 512 + col0:(t + 1) * 512],
                            start=True, stop=(o < 0))
                        if o >= 0:
                            # after the col0 shift the partial block is always
                            # the i' >= j triangle; accumulate the additive
                            # mask with an fp8-DR matmul right behind scores
                            nc.tensor.matmul(ps_sc[:, i, col0:col0 + P], ident8,
                                             mask8, start=False, stop=True,
                                             perf_mode=DR)
                    a0, a1 = col0s
                    # one exp instruction across both psum banks; for a
                    # diagonal pair the flat range [a0:1024] includes the
                    # stale segment [512:512+a1) -- exp of an old (finite)
                    # score lands in a pt region that nothing reads
                    nc.scalar.activation(
                        pt.rearrange("p a b -> p (a b)")[:, a0:1024],
                        ps_sc.rearrange("p a b -> p (a b)")[:, a0:1024],
                        AF.Exp, bias=EXPB)
                    # fp16 pair-sum for the denominator tree (DVE 2x); for
                    # the diagonal pairs only the region >= a0 is live, and
                    # [a0, a1) has just the first chunk
                    s01 = s1pool.tile([P, 512], F16, tag="s01")
                    if a1 > a0:
                        nc.vector.tensor_copy(s01[:, a0:a1], pt[:, 0, a0:a1])
                        nc.vector.tensor_add(s01[:, a1:512], pt[:, 0, a1:512],
                                             pt[:, 1, a1:512])
                    else:
                        nc.vector.tensor_add(s01, pt[:, 0, :], pt[:, 1, :])
                    live[idx] = (pt, col0s, s01)

                for i in range(min(DEPTH, len(items))):
                    front(i)
                ys = {}
                s01s = {}
                q4s = {}
                diaga = {}
                dfirst = {}
                for idx, (h, pr) in enumerate(items):
                    if idx + DEPTH < len(items):
                        front(idx + DEPTH)
                    g = h // NREP
                    c0 = 2 * pr
                    if pr == 0:
                        ps_y = psy.tile([P, 512], F32, tag="y")
                        ps_s = pss.tile([P, 512], F32, tag="s")
                        ys[h] = (ps_y, ps_s)
                    ps_y, ps_s = ys[h]
                    pt, col0s, s01 = live.pop(idx)
                    for i, c in enumerate((c0, c0 + 1)):
                        col0 = col0s[i]
                        nc.tensor.matmul(ps_y[:, col0:512],
                                         v_sb[:, c, g * P:(g + 1) * P],
                                         pt[:, i, col0:512],
                                         start=(pr == 0 and i == 0),
                                         stop=(pr == npair - 1 and i == 1))
                    # denominator tree: full pairs combine to quads on the
                    # Pool engine (one ones-matmul per quad); the two
                    # diagonal pairs feed the ones-matmul directly
                    nq = npair - 2  # full pairs; always even

                    def den_mm(src, stop):
                        nc.tensor.matmul(ps_s, ones_mat, src,
                                         start=dfirst.pop(h, True), stop=stop)

                    if pr < nq:
                        if pr % 2 == 0:
                            s01s[h] = s01
                        else:
                            # quad, then oct (fp16 tree on the DVE): each
                            # extra level halves the ones-matmul PE cost
                            q4 = bpool.tile([P, 512], F16, tag="q4")
                            nc.vector.tensor_tensor(q4, s01s.pop(h), s01,
                                                    ALU.add)
                            ql = q4s.setdefault(h, [])
                            ql.append(q4)
                            if len(ql) == 2:
                                o8 = bpool.tile([P, 512], F16, tag="q4")
                                nc.vector.tensor_add(o8, ql[0], ql[1])
                                ql.clear()
                                den_mm(o8, False)
                                dfirst[h] = False
                    elif pr == nq:
                        diaga[h] = s01
                    else:
                        # merge the second diagonal pair's live region into
                        # the first, then one ones-matmul covers both
                        sA = diaga.pop(h)
                        nc.vector.tensor_add(sA[:, 256:512], sA[:, 256:512],
                                             s01[:, 256:512])
                        for q4 in q4s.pop(h, []):
                            den_mm(q4, False)
                            dfirst[h] = False
                        den_mm(sA, True)
                    if pr == npair - 1:
                        # normalize + split into the fp8 hi/lo
                        # output-projection operand (den is already
                        # replicated across partitions by ones_mat)
                        rc = rpool.tile([P, 512], F32, tag="rc")
                        nc.vector.reciprocal(rc, ps_s)
                        yn = bpool.tile([P, 512], BF16, tag="yn")
                        nc.vector.scalar_tensor_tensor(
                            yn, ps_y, YS, rc, op0=ALU.mult, op1=ALU.mult)
                        yh = y8h[:, h, t * 512:(t + 1) * 512]
                        nc.vector.tensor_copy(yh, yn)
                        nc.vector.scalar_tensor_tensor(
                            y8l[:, h, t * 512:(t + 1) * 512], yn, 1.0, yh,
                            op0=ALU.mult, op1=ALU.subtract)
                        # fill the head-boundary psum-chain drain with two
                        # output-projection tiles of the previous tq-tile
                        for _ in range(2):
                            if cq:
                                tc_, ct = cq.pop(0)
                                emit_c(tc_, ct)

            # tail: the final processed tq-tile's projection tiles
            while cq:
                tc_, ct = cq.pop(0)
                emit_c(tc_, ct, alt=True)
            tlast = TORD[-1]
            ctail = [(tc_, ct) for tc_ in range(4 * tlast, 4 * tlast + 4)
                     for ct in range(C // 512)]
            for tc_, ct in ctail:
                emit_c(tc_, ct, alt=True,
                       last=((tc_, ct) == ctail[-1]))

    nc.compile()
    return nc


_NC_CACHE = []


def _get_prog():
    if not _NC_CACHE:
        _NC_CACHE.append(_build())
    return _NC_CACHE[0]


def _split8(a, axis):
    """Stack (hi, lo) fp8 split along a new axis."""
    e4 = ml_dtypes.float8_e4m3
    hi = a.astype(e4)
    lo = (a - hi.astype(np.float32)).astype(e4)
    return np.ascontiguousarray(np.stack([hi, lo], axis=axis))


def _make_in_maps(inputs):
    x, cos, sin = inputs["x"], inputs["cos"], inputs["sin"]
    wq, wk, wv, wproj = inputs["wq"], inputs["wk"], inputs["wv"], inputs["wproj"]
    bf = ml_dtypes.bfloat16
    # [p, tc, d] tiling (contiguous DMA rows)
    cos2 = np.ascontiguousarray(
        cos.reshape(TOKCH, P, D // 2).transpose(1, 0, 2)).astype(bf)
    sin2 = np.ascontiguousarray(
        sin.reshape(TOKCH, P, D // 2).transpose(1, 0, 2)).astype(bf)
    in_maps = []
    for core in range(8):
        b, g = core // 2, core % 2
        qs = slice(g * QC, (g + 1) * QC)
        ks = slice(g * KC, (g + 1) * KC)
        # x[b].T is [C, T]; tile to [tokch, p(C-chunk), co, tk]
        xtb = (x[b].T.astype(np.float32)
               .reshape(C // P, P, TOKCH, P)     # [co, p, tc, tk]
               .transpose(2, 1, 0, 3))           # [tc, p, co, tk]
        wq2 = _split8(np.ascontiguousarray(wq[:, qs]) * WS, axis=1)
        wkv2 = _split8(np.hstack([wk[:, ks], wv[:, ks]]) * WS, axis=1)
        wp2 = _split8(np.ascontiguousarray(wproj[qs, :]) * WS, axis=1)
        in_maps.append({
            "xt8": _split8(xtb, axis=2),                 # [tc, p, 2, co, tk]
            # [ch, C, 2, 512] / [kv, C, 2, KC] / [ct, QC, 2, 512]
            "wq8": np.ascontiguousarray(
                wq2.reshape(C, 2, 2, 512).transpose(2, 0, 1, 3)),
            "wkv8": np.ascontiguousarray(
                wkv2.reshape(C, 2, 2, KC).transpose(2, 0, 1, 3)),
            "wp8": np.ascontiguousarray(
                wp2.reshape(QC, 2, 4, 512).transpose(2, 0, 1, 3)),
            "cosd": cos2,
            "sind": sin2,
        })
    return in_maps


def kernel(x, cos, sin, wq, wk, wv, wproj):
    nc = _get_prog()
    in_maps = _make_in_maps(dict(x=x, cos=cos, sin=sin, wq=wq, wk=wk, wv=wv, wproj=wproj))
    res = run_bass_kernel_spmd(nc, in_maps, core_ids=list(range(8))).results
    outp = np.empty((B, T, C), np.float32)
    for b in range(B):
        outp[b] = res[2 * b]["out"] + res[2 * b + 1]["out"]
    return outp


# revision 84
# speedup vs baseline: 1.2505x; 1.0036x over previous
"""Causal self-attention (GQA + RoPE + QK-norm) Trainium2 Bass kernel.

Sharding: 8 cores = 4 batches x 2 head-groups.  Core c -> batch c//2,
q heads (c%2)*8..+8, kv heads (c%2)*2..+2.  wproj is row-sharded, so each
core emits a partial (T, C) output; the host sums the two partials per batch.

Device-side layout strategy (per core):
  - x and the projection weights ship as error-compensated fp8 hi/lo pairs
    (hi = fp8(v), lo = fp8(v - hi)); the QKV and output projections run as
    3-pass DoubleRow fp8 matmuls (hi*hi + hi*lo + lo*hi, dropping the
    ~0.4%-scale lo*lo term) -- 4/3x faster than bf16 on the PE.
  - QKV projections produce Q,K token-major.  rms-norm runs BEFORE rope
    (rope is a rotation per frequency pair, so it preserves the per-head
    norm): the psum evacuation applies the per-head 1/rms as the Act-copy
    scale, then the rope runs as an all-bf16 DVE chain (2x mode), and the
    128x128 PE transposes produce qT/kT feature-major.  V is token-major
    fp16, which is exactly the p@v stationary layout.
  - scores are computed transposed (scoresT[tk, tq]) in bf16, psums in
    2-chunk pairs so one exp covers 1024 columns; exp applies bias -4 and
    writes fp16 p-tiles (fits fp16 range since qk-norm bounds |s|<=11.32;
    the bias cancels between numerator and denominator).
  - softmax denominator: fp16 pair-sums on the DVE + quad-sums on the Pool
    engine feed a ones-column matmul per quad -- the PE streams den at 1/4
    of the direct cost.
  - output projection: yT is normalized and split into fp8 hi/lo during
    phase B; 3-pass fp8 DoubleRow over hc pairs.  Partial written fp32.
"""

import numpy as np
import ml_dtypes
from contextlib import ExitStack

import concourse.bass as bass
import concourse.mybir as mybir
import concourse.tile as tile
from concourse import bacc
from concourse.bass_utils import run_bass_kernel_spmd
from concourse.masks import make_identity

BF16 = mybir.dt.bfloat16
F16 = mybir.dt.float16
F32 = mybir.dt.float32
F8E4 = mybir.dt.float8e4
DR = mybir.MatmulPerfMode.DoubleRow
AF = mybir.ActivationFunctionType
ALU = mybir.AluOpType

B, T, C = 4, 2048, 2048
H, KV, D = 16, 4, 128
HG, KVG = H // 2, KV // 2          # per-core q heads (8), kv heads (2)
QC, KC = HG * D, KVG * D           # 1024, 256
P = 128
TOKCH = T // P                     # 16 token chunks
NREP = H // KV                     # 4
EPS = 1e-5
NEG = -30.0                        # additive causal mask: qk-norm bounds
                                   # |s|<=11.32, so exp(s+NEG-4) is <=1e-10
                                   # of any unmasked term; -30 is exact in
                                   # e4m3 so the mask-add can run as an fp8
                                   # DoubleRow matmul at half cost
WS = 32.0                          # fp8 weight pre-scale (host): keeps w
                                   # entries in e4m3 normal range; q/k are
                                   # scale-invariant through rms-norm, v is
                                   # rescaled by 1/WS at psum evacuation
YS = 8.0                           # y pre-scale for the fp8 hi/lo split of
                                   # the output-projection LHS
EXPB = -4.0                        # exp(s-4) so p fits fp16 range; cancels
                                   # between numerator and denominator


DEBUG_DUMP = False
PHASES = ("A", "B", "C")


def _build():
    nc = bacc.Bacc("TRN2", target_bir_lowering=False, debug=False, num_devices=8)
    # x pre-tiled by the host as [tokch, p, co, tk] so every DMA partition row
    # is contiguous; hi/lo fp8 pair (combined traffic = bf16 baseline)
    # hi/lo fp8 pairs ship interleaved in one tensor per operand: one DMA
    # carries both (the HWDGE issue sequencer at ~625ns/DMA is the startup
    # bottleneck, so DMA count matters more than size)
    xt8 = nc.dram_tensor("xt8", [TOKCH, P, 2, C // P, P], F8E4,
                         kind="ExternalInput")
    # outer dims split by first use so every DMA slice stays <=3 dims:
    # wq8 by column half, wkv8 by K/V, wp8 by ct chunk
    wq8 = nc.dram_tensor("wq8", [2, C, 2, 512], F8E4, kind="ExternalInput")
    wkv8 = nc.dram_tensor("wkv8", [2, C, 2, KC], F8E4, kind="ExternalInput")
    wp8 = nc.dram_tensor("wp8", [4, QC, 2, 512], F8E4, kind="ExternalInput")
    # cos/sin pre-tiled by host as [p, tc, d] bf16 (keeps the rope chain in
    # the DVE 2x mode)
    cosd = nc.dram_tensor("cosd", [P, TOKCH, D // 2], BF16, kind="ExternalInput")
    sind = nc.dram_tensor("sind", [P, TOKCH, D // 2], BF16, kind="ExternalInput")
    out = nc.dram_tensor("out", [T, C], F32, kind="ExternalOutput")

    with tile.TileContext(nc) as tc, ExitStack() as ctx:
        singles = ctx.enter_context(tc.tile_pool(name="singles", bufs=1))
        # bufs must cover the V-lag window (xtile(t) is re-read by the lagged
        # V projection at iteration t+VLAG); the pool closes with phase A
        phase_a_pools = ExitStack()
        xpool = phase_a_pools.enter_context(tc.tile_pool(name="xa", bufs=8))

        # ---- resident tensors ----
        wq_sb = singles.tile([P, 2, C // P, 2, 512], F8E4)
        wkv_sb = singles.tile([P, 2, C // P, 2, KC], F8E4)
        wqr = wq8.rearrange("ch (co p) two q -> p ch co two q", p=P)
        wkvr = wkv8.rearrange("kv (co p) two q -> p kv co two q", p=P)
        cos_sb = singles.tile([P, TOKCH, D // 2], BF16)
        sin_sb = singles.tile([P, TOKCH, D // 2], BF16)
        # x hi/lo tiles for iteration 0 go out first (they gate the PE),
        # then weights in consumption order (first q pair, k halves, q bulk,
        # V last -- first consumed at iteration VLAG)
        xtile0 = xpool.tile([P, 2, C // P, P], F8E4, tag="xt")
        nc.sync.dma_start(xtile0[:, :, 0:4, :], xt8[0, :, :, 0:4, :])
        # weight DMAs in exact consumption order (co-major, q columns
        # before k); hi+lo ride together so the issue count stays low
        qs2 = [nc.sync, nc.scalar]
        for i, co in enumerate(range(0, C // P, 4)):
            qs2[i % 2].dma_start(wq_sb[:, 0, co:co + 4], wqr[:, 0, co:co + 4])
        nc.scalar.dma_start(xtile0[:, :, 4:16, :], xt8[0, :, :, 4:16, :])
        for i, co in enumerate(range(0, C // P, 8)):
            qs2[i % 2].dma_start(wq_sb[:, 1, co:co + 8], wqr[:, 1, co:co + 8])
        for i, co in enumerate(range(0, C // P, 8)):
            qs2[(i + 1) % 2].dma_start(wkv_sb[:, 0, co:co + 8],
                                       wkvr[:, 0, co:co + 8])
        # x tiles for t=1,2 ahead of the V-weight bulk
        xtile_pre = {}
        for tp in (1, 2):
            xp = xpool.tile([P, 2, C // P, P], F8E4, tag="xt")
            qs2[tp % 2].dma_start(xp, xt8[tp])
            xtile_pre[tp] = xp
        nc.gpsimd.dma_start(cos_sb, cosd[:])
        nc.gpsimd.dma_start(sin_sb, sind[:])

        ident = singles.tile([P, P], BF16)
        make_identity(nc, ident)
        # ones MATRIX: stationary for the denominator matmuls -> the psum
        # output carries the tk-sum replicated across all 128 partitions
        # (same cost as a 1-row output; kills the partition_broadcast)
        ones_mat = singles.tile([P, P], F16)
        nc.vector.memset(ones_mat, 1.0)
        zero_col = singles.tile([P, 1], F32)
        nc.vector.memset(zero_col, 0.0)
        eps_col = singles.tile([P, 1], F32)
        nc.vector.memset(eps_col, EPS)
        nb_col = singles.tile([P, 1], F32)
        nc.vector.memset(nb_col, EXPB)
        nc.const_aps.aps[(F32, 0.0)] = zero_col[:]
        nc.const_aps.aps[(F32, EPS)] = eps_col[:]
        nc.const_aps.aps[(F32, EXPB)] = nb_col[:]
        # scratch for the dummy exp that prewarms the exp act-table at the
        # A->B phase boundary (overlaps the 1.28us table load)
        warm = singles.tile([1, 1], F32)

        # diagonal-block mask: keep where i >= j (j = tk partition, i = tq
        # free), ADDed into the scores psum by a matmul.  fp8 DoubleRow pair
        # (identity, 0) x (mask, 0) runs the add at 0.5 cyc/col.
        ident8 = singles.tile([P, 2, P], F8E4)
        nc.vector.memset(ident8, 0.0)
        nc.vector.tensor_copy(ident8[:, 0, :], ident)
        mask8 = singles.tile([P, 2, P], F8E4)
        nc.vector.memset(mask8, 0.0)
        nc.gpsimd.affine_select(
            out=mask8[:, 0, :], in_=mask8[:, 0, :],
            compare_op=ALU.is_ge, fill=NEG,
            base=0, pattern=[[1, P]], channel_multiplier=-1,
        )

        qT = singles.tile([P, HG, T], BF16)      # [d, h, tok]
        kT = singles.tile([P, KVG, T], BF16)
        v_sb = singles.tile([P, TOKCH, KC], F16)  # [tok%128, chunk, vcol]
        # output-projection LHS, fp8 hi/lo split of yT*YS (written in B)
        y8h = singles.tile([P, HG, T], F8E4)
        y8l = singles.tile([P, HG, T], F8E4)

        # ================= phase A: QKV proj + norm + RoPE ================
        if "A" not in PHASES:
            pass
        else:
         with phase_a_pools, \
             tc.tile_pool(name="pa", bufs=2, space="PSUM") as pps, \
             tc.tile_pool(name="pkv", bufs=1, space="PSUM") as pkv, \
             tc.tile_pool(name="sa", bufs=3) as spool:
            VLAG = 6
            nco = C // P
            xtiles = {}
            # transposes lag one iteration behind their rope chain so they
            # never sit dep-blocked in the PE's 4-deep wait queue
            pending_tr = []

            def flush_trs(upto=None):
                while pending_tr and (upto is None or pending_tr[0][4] <= upto):
                    qbf, dstT, h0, nh, tt = pending_tr.pop(0)
                    pst = pkv.tile([P, 4, P], BF16, tag="tr")
                    for i in range(nh):
                        nc.tensor.transpose(pst[:, i, :], qbf[:, i, :], ident)
                    nc.scalar.copy(
                        dstT[:, h0:h0 + nh, tt * P:(tt + 1) * P], pst[:, 0:nh, :])

            def mm3(ps, xt, w_sb, half):
                # 3-pass hi/lo fp8 DoubleRow accumulation, co-major so the
                # weight consumption follows the startup DMA stream order
                for co in range(0, nco, 2):
                    for pi, (ai, bi) in enumerate(((0, 0), (0, 1), (1, 0))):
                        nc.tensor.matmul(
                            ps, xt[:, ai, co:co + 2, :],
                            w_sb[:, half, co:co + 2, bi, :],
                            start=(pi == 0 and co == 0),
                            stop=(pi == 2 and co == nco - 2),
                            perf_mode=DR)

            def v_chunk(tv):
                xv = xtiles.pop(tv)
                ps_v = pkv.tile([P, KC], F32, tag="v")
                mm3(ps_v, xv, wkv_sb, 1)
                # cast straight to resident token-major fp16 buffer; 1/WS
                # undoes the host-side fp8 weight pre-scale
                nc.scalar.mul(v_sb[:, tv, :], ps_v, 1.0 / WS)

            for t in range(TOKCH):
                if t == 0:
                    xtile = xtile0
                elif t in xtile_pre:
                    xtile = xtile_pre[t]
                else:
                    xtile = xpool.tile([P, 2, C // P, P], F8E4, tag="xt")
                    (nc.sync if t % 2 == 0 else nc.scalar).dma_start(
                        xtile, xt8[t])
                xtiles[t] = xtile
                ps_q0 = pps.tile([P, 512], F32, tag="q0")
                ps_q1 = pps.tile([P, 512], F32, tag="q1")
                ps_k = pkv.tile([P, KC], F32, tag=("k" if t % 2 == 0 else "k2"))

                # rms-norm stats run off the psum (pre-rope: rope is a
                # rotation per frequency pair so it preserves per-head
                # norms); the rope chain itself runs on an all-bf16
                # evacuation (DVE 2x mode), and the 1/rms multiplier is
                # applied once at the end.
                def norm_rope(ps, nh, dstT, h0, qscale):
                    h2 = D // 2
                    psv = ps.rearrange("p (h d) -> p h d", h=nh)
                    sq = spool.tile([P, nh, D], BF16, tag=f"sq{h0}_{nh}")
                    nc.scalar.activation(sq, psv, AF.Square)
                    ss = spool.tile([P, nh], BF16, tag=f"ss{h0}_{nh}")
                    with nc.allow_low_precision(
                            reason="rms stats; 0.4% scale err is fine"):
                        nc.vector.tensor_reduce(ss, sq,
                                                axis=mybir.AxisListType.X,
                                                op=ALU.add)
                    rt = spool.tile([P, nh], F32, tag=f"rt{h0}_{nh}")
                    nc.scalar.activation(rt, ss, AF.Sqrt, scale=1.0 / D,
                                         bias=EPS)
                    rq = spool.tile([P, nh], F32, tag=f"rq{h0}_{nh}")
                    nc.vector.reciprocal(rq, rt)
                    if qscale != 1.0:
                        nc.vector.tensor_scalar_mul(rq, rq, qscale)
                    qe = spool.tile([P, nh, D], BF16, tag=f"qe{h0}_{nh}")
                    nc.scalar.copy(qe, psv)
                    # rope on the bf16 evacuation (all-2-byte DVE ops)
                    q1, q2 = qe[:, :, 0:h2], qe[:, :, h2:D]
                    r = spool.tile([P, nh, 2, h2], BF16, tag=f"ro{h0}_{nh}")
                    r1, r2 = r[:, :, 0, :], r[:, :, 1, :]
                    s2 = spool.tile([P, nh, h2], BF16, tag=f"sc{h0}_{nh}")
                    cs = cos_sb[:, t, None, :].to_broadcast([P, nh, h2])
                    sn = sin_sb[:, t, None, :].to_broadcast([P, nh, h2])
                    nc.vector.tensor_mul(r1, q1, cs)
                    nc.vector.tensor_mul(s2, q2, sn)
                    nc.vector.tensor_sub(r1, r1, s2)
                    nc.vector.tensor_mul(r2, q1, sn)
                    nc.vector.tensor_mul(s2, q2, cs)
                    nc.vector.tensor_add(r2, r2, s2)
                    rf = r.rearrange("p h a d -> p h (a d)")
                    qbf = spool.tile([P, nh, D], BF16, tag=f"qb{h0}_{nh}")
                    nc.vector.tensor_mul(
                        qbf, rf, rq[:, :, None].to_broadcast([P, nh, D]))
                    pending_tr.append((qbf, dstT, h0, nh, t))

                qsc = 1.0 / float(np.sqrt(D))
                mm3(ps_q0, xtile, wq_sb, 0)
                mm3(ps_q1, xtile, wq_sb, 1)
                mm3(ps_k, xtile, wkv_sb, 0)
                norm_rope(ps_q0, 4, qT, 0, qsc)
                norm_rope(ps_q1, 4, qT, 4, qsc)
                norm_rope(ps_k, KVG, kT, 0, 1.0)
                if t == 2:
                    # V weights: first consumed at t=VLAG; issuing here keeps
                    # their transfers out of the startup-critical DMA window
                    for co in range(0, C // P, 4):
                        nc.gpsimd.dma_start(wkv_sb[:, 1, co:co + 4],
                                            wkvr[:, 1, co:co + 4])
                if t >= VLAG:
                    v_chunk(t - VLAG)
                # transposes lag two iterations: their rope chains (DVE) are
                # certainly drained, so they never block the PE stream
                flush_trs(upto=t - 2)
                if t == TOKCH - 1:
                    # prewarm the exp act-table; the 1.28us load runs behind
                    # the V tail
                    nc.scalar.activation(warm, zero_col[0:1, :], AF.Exp)

            # lagged V tail: pure PE work that covers the final rope chains
            for tv in range(TOKCH - VLAG, TOKCH):
                v_chunk(tv)
                if tv == TOKCH - 2:
                    flush_trs()

        # ================= phase B: attention ============================
        # wp prefetch: issue at phase-B start so the tiles are resident long
        # before phase C begins (phase-A pools have closed, SBUF is free)
        wpool = ctx.enter_context(tc.tile_pool(name="wp", bufs=1))
        wpr = wp8.rearrange("ct (hc p) two c -> p ct hc two c", p=P)
        wp_ts = []
        for ct in range(C // 512):
            wp_t = wpool.tile([P, HG, 2, 512], F8E4, tag=f"wpt{ct}")
            (nc.sync if ct % 2 == 0 else nc.scalar).dma_start(
                wp_t, wpr[:, ct])
            wp_ts.append(wp_t)

        if "B" not in PHASES:
            pass
        else:
         with tc.tile_pool(name="psc", bufs=2, space="PSUM") as psc, \
             tc.tile_pool(name="psy", bufs=2, space="PSUM") as psy, \
             tc.tile_pool(name="pss", bufs=1, space="PSUM") as pss, \
             tc.tile_pool(name="po", bufs=1, space="PSUM") as pso, \
             tc.tile_pool(name="pb", bufs=8) as ppool, \
             tc.tile_pool(name="sb", bufs=4) as bpool, \
             tc.tile_pool(name="s1", bufs=6) as s1pool, \
             tc.tile_pool(name="sr", bufs=2) as rpool, \
             tc.tile_pool(name="so", bufs=3) as opool:
            NT = T // 512  # 4 tq tiles
            OSC = 1.0 / (YS * WS)
            cq = []  # pending output-projection tiles, emitted at head
            #          boundaries to fill the PE while psum chains drain
            ci = 0

            def emit_c(tc_, ct, last=False, alt=False):
                nonlocal ci
                wp_t = wp_ts[ct]
                # in the tail (alt=True) the psy ring is idle: alternate
                # between the pso bank and psy's so back-to-back projection
                # tiles don't serialize on one psum evacuation
                if alt and ci % 2 == 0:
                    ps_o = psy.tile([P, 512], F32, tag="y")
                else:
                    ps_o = pso.tile([P, 512], F32, tag="o")
                ob = opool.tile([P, 512], F32, tag="ob")
                tsl = slice(tc_ * P, (tc_ + 1) * P)

                def proj(cs, ps):
                    for pi, (a, bi) in enumerate(
                            ((y8h, 0), (y8h, 1), (y8l, 0))):
                        for hc in range(0, HG, 2):
                            nc.tensor.matmul(
                                ps, a[:, hc:hc + 2, tsl],
                                wp_t[:, hc:hc + 2, bi, cs],
                                start=(pi == 0 and hc == 0),
                                stop=(pi == 2 and hc == HG - 2),
                                perf_mode=DR)

                if not last:
                    proj(slice(0, 512), ps_o)
                    ci += 1
                    if ci % 2 == 0:
                        nc.vector.tensor_scalar_mul(ob, ps_o, OSC)
                    else:
                        nc.scalar.mul(ob, ps_o, OSC)
                    (nc.sync if ci % 2 == 0 else nc.scalar).dma_start(
                        out[tc_ * P:(tc_ + 1) * P, ct * 512:(ct + 1) * 512], ob)
                else:
                    # final tile in two pipelined halves to shorten the
                    # copy->dma drain tail
                    for q in range(2):
                        cs = slice(q * 256, (q + 1) * 256)
                        proj(cs, ps_o[:, cs])
                        if q == 0:
                            nc.vector.tensor_scalar_mul(ob[:, cs],
                                                        ps_o[:, cs], OSC)
                        else:
                            nc.scalar.mul(ob[:, cs], ps_o[:, cs], OSC)
                        (nc.sync if q == 0 else nc.scalar).dma_start(
                            out[tc_ * P:(tc_ + 1) * P,
                                ct * 512 + q * 256:ct * 512 + (q + 1) * 256],
                            ob[:, cs])
            # software pipeline over chunk PAIRS: scores+exp for pair
            # idx+DEPTH are emitted before pv of pair idx, so the
            # scores->mask->exp chain hides behind PE work.
            DEPTH = 3
            # tile order: start with a mid-length tile so the first tile's
            # head boundaries are not too short, then feed each finished
            # tile's projection tiles into the next tile's head boundaries
            # (the short-head t=0/1 tiles get C-fill this way too)
            TORD = [0, 1, 2, 3]
            for ti, t in enumerate(TORD):
                nch = 4 * (t + 1)
                npair = nch // 2
                items = [(h, pr) for h in range(HG) for pr in range(npair)]
                live = {}
                # previously finished tq-tile's projection tiles
                if ti >= 1:
                    tprev = TORD[ti - 1]
                    cq.extend((tc_, ct) for tc_ in range(4 * tprev,
                                                         4 * tprev + 4)
                              for ct in range(C // 512))

                def front(idx):
                    h, pr = items[idx]
                    g = h // NREP
                    c0 = 2 * pr
                    ps_sc = psc.tile([P, 2, 512], F32, tag="sc")
                    pt = ppool.tile([P, 2, 512], F16, tag="pt")
                    col0s = []
                    for i, c in enumerate((c0, c0 + 1)):
                        o = c * P - t * 512
                        col0 = max(o, 0)
                        col0s.append(col0)
                        nc.tensor.matmul(
                            ps_sc[:, i, col0:512], kT[:, g, c * P:(c + 1) * P],
                            qT[:, h, t * 512 + col0:(t + 1) * 512],
                            start=True, stop=(o < 0))
                        if o >= 0:
                            # after the col0 shift the partial block is always
                            # the i' >= j triangle; accumulate the additive
                            # mask with an fp8-DR matmul right behind scores
                            nc.tensor.matmul(ps_sc[:, i, col0:col0 + P], ident8,
                                             mask8, start=False, stop=True,
                                             perf_mode=DR)
                    a0, a1 = col0s
                    # one exp instruction across both psum banks; for a
                    # diagonal pair the flat range [a0:1024] includes the
                    # stale segment [512:512+a1) -- exp of an old (finite)
                    # score lands in a pt region that nothing reads
                    nc.scalar.activation(
                        pt.rearrange("p a b -> p (a b)")[:, a0:1024],
                        ps_sc.rearrange("p a b -> p (a b)")[:, a0:1024],
                        AF.Exp, bias=EXPB)
                    # fp16 pair-sum for the denominator tree (DVE 2x); for
                    # the diagonal pairs only the region >= a0 is live, and
                    # [a0, a1) has just the first chunk
                    s01 = s1pool.tile([P, 512], F16, tag="s01")
                    if a1 > a0:
                        nc.vector.tensor_copy(s01[:, a0:a1], pt[:, 0, a0:a1])
                        nc.vector.tensor_add(s01[:, a1:512], pt[:, 0, a1:512],
                                             pt[:, 1, a1:512])
                    else:
                        nc.vector.tensor_add(s01, pt[:, 0, :], pt[:, 1, :])
                    live[idx] = (pt, col0s, s01)

                for i in range(min(DEPTH, len(items))):
                    front(i)
                ys = {}
                s01s = {}
                q4s = {}
                diaga = {}
                for idx, (h, pr) in enumerate(items):
                    if idx + DEPTH < len(items):
                        front(idx + DEPTH)
                    g = h // NREP
                    c0 = 2 * pr
                    if pr == 0:
                        ps_y = psy.tile([P, 512], F32, tag="y")
                        ps_s = pss.tile([P, 512], F32, tag="s")
                        ys[h] = (ps_y, ps_s)
                    ps_y, ps_s = ys[h]
                    pt, col0s, s01 = live.pop(idx)
                    for i, c in enumerate((c0, c0 + 1)):
                        col0 = col0s[i]
                        nc.tensor.matmul(ps_y[:, col0:512],
                                         v_sb[:, c, g * P:(g + 1) * P],
                                         pt[:, i, col0:512],
                                         start=(pr == 0 and i == 0),
                                         stop=(pr == npair - 1 and i == 1))
                    # denominator tree: full pairs combine to quads on the
                    # Pool engine (one ones-matmul per quad); the two
                    # diagonal pairs feed the ones-matmul directly
                    nq = npair - 2  # full pairs; always even

                    if pr < nq:
                        if pr % 2 == 0:
                            s01s[h] = s01
                        else:
                            # running fp16 tree on the DVE: fold every new
                            # quad into one pending node so the PE sees a
                            # single ones-matmul per (head, tq-tile)
                            q4 = bpool.tile([P, 512], F16, tag="q4")
                            nc.vector.tensor_tensor(q4, s01s.pop(h), s01,
                                                    ALU.add)
                            ql = q4s.setdefault(h, [])
                            ql.append(q4)
                            if len(ql) == 2:
                                o8 = bpool.tile([P, 512], F16, tag="q4")
                                nc.vector.tensor_add(o8, ql[0], ql[1])
                                ql.clear()
                                ql.append(o8)
                    elif pr == nq:
                        diaga[h] = s01
                    else:
                        # merge the second diagonal pair's live region and
                        # the pending full-pair node into the first diagonal
                        # tile, then one ones-matmul covers the whole head
                        sA = diaga.pop(h)
                        nc.vector.tensor_add(sA[:, 256:512], sA[:, 256:512],
                                             s01[:, 256:512])
                        for node in q4s.pop(h, []):
                            nc.vector.tensor_add(sA, sA, node)
                        nc.tensor.matmul(ps_s, ones_mat, sA,
                                         start=True, stop=True)
                    if pr == npair - 1:
                        # normalize + split into the fp8 hi/lo
                        # output-projection operand (den is already
                        # replicated across partitions by ones_mat)
                        rc = rpool.tile([P, 512], F32, tag="rc")
                        nc.vector.reciprocal(rc, ps_s)
                        yn = bpool.tile([P, 512], BF16, tag="yn")
                        nc.vector.scalar_tensor_tensor(
                            yn, ps_y, YS, rc, op0=ALU.mult, op1=ALU.mult)
                        yh = y8h[:, h, t * 512:(t + 1) * 512]
                        nc.vector.tensor_copy(yh, yn)
                        nc.vector.scalar_tensor_tensor(
                            y8l[:, h, t * 512:(t + 1) * 512], yn, 1.0, yh,
                            op0=ALU.mult, op1=ALU.subtract)
                        # fill the head-boundary psum-chain drain with two
                        # output-projection tiles of the previous tq-tile
                        for _ in range(2):
                            if cq:
                                tc_, ct = cq.pop(0)
                                emit_c(tc_, ct)

            # tail: the final processed tq-tile's projection tiles
            while cq:
                tc_, ct = cq.pop(0)
                emit_c(tc_, ct, alt=True)
            tlast = TORD[-1]
            ctail = [(tc_, ct) for tc_ in range(4 * tlast, 4 * tlast + 4)
                     for ct in range(C // 512)]
            for tc_, ct in ctail:
                emit_c(tc_, ct, alt=True,
                       last=((tc_, ct) == ctail[-1]))

    nc.compile()
    return nc


_NC_CACHE = []


def _get_prog():
    if not _NC_CACHE:
        _NC_CACHE.append(_build())
    return _NC_CACHE[0]


def _split8(a, axis):
    """Stack (hi, lo) fp8 split along a new axis."""
    e4 = ml_dtypes.float8_e4m3
    hi = a.astype(e4)
    lo = (a - hi.astype(np.float32)).astype(e4)
    return np.ascontiguousarray(np.stack([hi, lo], axis=axis))


def _make_in_maps(inputs):
    x, cos, sin = inputs["x"], inputs["cos"], inputs["sin"]
    wq, wk, wv, wproj = inputs["wq"], inputs["wk"], inputs["wv"], inputs["wproj"]
    bf = ml_dtypes.bfloat16
    # [p, tc, d] tiling (contiguous DMA rows)
    cos2 = np.ascontiguousarray(
        cos.reshape(TOKCH, P, D // 2).transpose(1, 0, 2)).astype(bf)
    sin2 = np.ascontiguousarray(
        sin.reshape(TOKCH, P, D // 2).transpose(1, 0, 2)).astype(bf)
    in_maps = []
    for core in range(8):
        b, g = core // 2, core % 2
        qs = slice(g * QC, (g + 1) * QC)
        ks = slice(g * KC, (g + 1) * KC)
        # x[b].T is [C, T]; tile to [tokch, p(C-chunk), co, tk]
        xtb = (x[b].T.astype(np.float32)
               .reshape(C // P, P, TOKCH, P)     # [co, p, tc, tk]
               .transpose(2, 1, 0, 3))           # [tc, p, co, tk]
        wq2 = _split8(np.ascontiguousarray(wq[:, qs]) * WS, axis=1)
        wkv2 = _split8(np.hstack([wk[:, ks], wv[:, ks]]) * WS, axis=1)
        wp2 = _split8(np.ascontiguousarray(wproj[qs, :]) * WS, axis=1)
        in_maps.append({
            "xt8": _split8(xtb, axis=2),                 # [tc, p, 2, co, tk]
            # [ch, C, 2, 512] / [kv, C, 2, KC] / [ct, QC, 2, 512]
            "wq8": np.ascontiguousarray(
                wq2.reshape(C, 2, 2, 512).transpose(2, 0, 1, 3)),
            "wkv8": np.ascontiguousarray(
                wkv2.reshape(C, 2, 2, KC).transpose(2, 0, 1, 3)),
            "wp8": np.ascontiguousarray(
                wp2.reshape(QC, 2, 4, 512).transpose(2, 0, 1, 3)),
            "cosd": cos2,
            "sind": sin2,
        })
    return in_maps


def kernel(x, cos, sin, wq, wk, wv, wproj):
    nc = _get_prog()
    in_maps = _make_in_maps(dict(x=x, cos=cos, sin=sin, wq=wq, wk=wk, wv=wv, wproj=wproj))
    res = run_bass_kernel_spmd(nc, in_maps, core_ids=list(range(8))).results
    outp = np.empty((B, T, C), np.float32)
    for b in range(B):
        outp[b] = res[2 * b]["out"] + res[2 * b + 1]["out"]
    return outp
